# revision 1
# baseline (speedup 1.0000x reference)
"""Trainium2 Bass kernel for nn_CPCModel (CPC-style NCE loss).

Strategy (8 NeuronCores, full inputs on every core, no collectives):

The reference's leave-one-out softmax pooling collapses algebraically:
    pooled[i] = (T - e_i * zt_i) / (S - e_i),  e = exp(s), S = sum(e), T = sum(e_j zt_j)
so the [B,B] pooling matrix is never materialized.  The loss needs only
    nce = -mean_i( total[i,i] - logsumexp_j total[i,j] )
with  total[i, j in group g] = Azw_g[i]·pooled_g[j] + Czw[i]·c[j] + delta_g[i]
where Azw_g = zw @ Ww_g, Czw = zw @ Wk_w, delta_g = zw @ (Ww_g_b + Wk_b).

Each core redundantly computes the cheap pooling prep for all 4096 rows
(no collectives) and computes its own 512 rows of the [4096,4096] total
matrix + row-wise sum(exp(total - 44)); the diagonal comes from an
elementwise product.  Host sums 8x[128,4] partial row values.

Dtypes: the big matmuls (U = [Czw;Azw_g] builds and the 512x4096 total)
run fp32r (full-rate, ~19-bit mantissa).  The small prep matmuls (zt, h,
s, broadcasts, delta, diag partition-sums) run bf16 — the fp32r ISA mode
requires 128 output partitions and even N, which those shapes violate.
Host does layout prep only (transposes / stacking of weights + zw/c).
"""

import numpy as np

import concourse.bacc as bacc
import concourse.bass as bass
import concourse.mybir as mybir
import concourse.tile as tile
from concourse.bass_utils import run_bass_kernel_spmd

N_CORES = 8
B = 4096
OWN = B // N_CORES            # 512 rows of `total` per core
G = 2048                      # group size
F32 = mybir.dt.float32
F32R = mybir.dt.float32r
BF16 = mybir.dt.bfloat16
AF = mybir.ActivationFunctionType
ALU = mybir.AluOpType
SHIFT = 44.0


def _r(ap):
    return ap.bitcast(F32R)


def _build_program(static_diag=False):
    nc = bacc.Bacc(
        "TRN2",
        target_bir_lowering=False,
        debug=False,
        num_devices=N_CORES,
    )

    def din(name, shape, dt):
        return nc.dram_tensor(name, shape, dt, kind="ExternalInput").ap()

    zwTb_d = din("zwTb", [128, B], BF16)     # concat(zw_0,zw_1).T in bf16
    zwoT_d = din("zwoT", [128, OWN], F32R)   # own 512 rows of zw, transposed
    zwoTb_d = din("zwoTb", [128, OWN], BF16)
    cT_d = din("cT", [64, B], F32R)          # c.T
    uw0_d = din("UW0", [128, 128], F32R)     # hstack(Wk_w, Ww0_w)
    uw1_d = din("UW1", [128, 128], F32R)     # hstack(Wk_w, Ww1_w)
    uwo_d = din("UWo", [128, 128], F32R)     # hstack(Wk_w, Ww_{g(core)})
    lwT0_d = din("lwT0", [128, 64], BF16)    # lin0_w.T
    lwT1_d = din("lwT1", [128, 64], BF16)    # lin1_w.T
    a1wB_d = din("a1wB", [128, 64], BF16)    # blockdiag(a0_1w.T, a1_1w.T)
    a2wB_d = din("a2wB", [64, 2], BF16)      # blockdiag(a0_2w.T, a1_2w.T)
    b0_d = din("b0", [128, 1], BF16)         # Ww0_b + Wk_b
    b1_d = din("b1", [128, 1], BF16)         # Ww1_b + Wk_b
    bo_d = din("b_own", [128, 1], BF16)      # b_{group(core)}
    sel2_d = din("sel2", [2, 128], BF16)     # [[1]*64+[0]*64, [0]*64+[1]*64]
    ones_d = din("ones", [128, 1], BF16)
    linb2_d = din("linb2", [128, 1], F32)    # [lin0_b ; lin1_b]
    a1b2_d = din("a1b2", [64, 1], F32)       # [a0_1b ; a1_1b]
    v_d = nc.dram_tensor("v", [128, 4], F32, kind="ExternalOutput").ap()

    from contextlib import ExitStack
    with tile.TileContext(nc) as tc, ExitStack() as ctx:
        pers = ctx.enter_context(tc.tile_pool(name="pers", bufs=1))
        scr = ctx.enter_context(tc.tile_pool(name="scr", bufs=2))
        pbig = ctx.enter_context(tc.tile_pool(name="pbig", bufs=2, space="PSUM"))
        psml = ctx.enter_context(tc.tile_pool(name="psml", bufs=3, space="PSUM"))
        ptin = ctx.enter_context(tc.tile_pool(name="ptin", bufs=1, space="PSUM"))

        def load(name, shape, src, dt):
            t = pers.tile(shape, dt, tag=name, name=name)
            nc.sync.dma_start(t[:], src[:])
            return t

        zwTb = load("zwTb", [128, B], zwTb_d, BF16)
        zwoT = load("zwoT", [128, OWN], zwoT_d, F32R)
        zwoTb = load("zwoTb", [128, OWN], zwoTb_d, BF16)
        uw0_s = load("uw0_s", [128, 128], uw0_d, F32R)
        uw1_s = load("uw1_s", [128, 128], uw1_d, F32R)
        uwo_s = load("uwo_s", [128, 128], uwo_d, F32R)
        lwT0 = load("lwT0", [128, 64], lwT0_d, BF16)
        lwT1 = load("lwT1", [128, 64], lwT1_d, BF16)
        lwT = [lwT0, lwT1]
        a1wB = load("a1wB", [128, 64], a1wB_d, BF16)
        a2wB = load("a2wB", [64, 2], a2wB_d, BF16)
        b0_s = load("b0_s", [128, 1], b0_d, BF16)
        b1_s = load("b1_s", [128, 1], b1_d, BF16)
        bo_s = load("bo_s", [128, 1], bo_d, BF16)
        sel2 = load("sel2", [2, 128], sel2_d, BF16)
        ones = load("ones", [128, 1], ones_d, BF16)
        linb2 = load("linb2", [128, 1], linb2_d, F32)
        a1b2 = load("a1b2", [64, 1], a1b2_d, F32)

        # ---------- V [128, 4096]: rows 0:64 = cT (direct), 64:128 = pooledT ----------
        V = pers.tile([128, B], F32R, tag="V")
        nc.sync.dma_start(V[0:64, :], cT_d[:])

        # ---------- U_g = [Czw ; Azw_g] via one stacked-weight matmul each ----------
        U0 = pers.tile([128, OWN], F32R, tag="U0")
        U1 = pers.tile([128, OWN], F32R, tag="U1")
        UOwn = pers.tile([128, OWN], F32R, tag="UOwn")
        for U, uw in [(U0, uw0_s), (U1, uw1_s), (UOwn, uwo_s)]:
            pu = psml.tile([128, 512], F32, tag="ps")
            nc.tensor.matmul(pu[:], uw[:], zwoT[:], start=True, stop=True)
            nc.vector.tensor_copy(U[:], pu[:])

        # ---------- delta bias columns: biasS[:, g*4+ic] = zw_own[ic]·b_g - SHIFT ----------
        biasS = pers.tile([128, 8], F32, tag="biasS")
        for g, bg in enumerate([b0_s, b1_s]):
            for ic in range(4):
                pd = ptin.tile([128, 1], F32, tag="pt")
                nc.tensor.matmul(pd[:], zwoTb[:, ic * 128:(ic + 1) * 128], bg[:],
                                 start=True, stop=True)
                nc.scalar.activation(biasS[:, g * 4 + ic:g * 4 + ic + 1], pd[:],
                                     AF.Copy, bias=-SHIFT)

        # ---------- ztT2 [128, 2048] bf16: zt0T on 0:64, zt1T on 64:128 ----------
        ztT2 = pers.tile([128, G], BF16, tag="ztT2")
        for ch in range(4):
            pz = psml.tile([128, 512], F32, tag="ps")
            sl = slice(ch * 512, (ch + 1) * 512)
            nc.tensor.matmul(pz[0:64, :], lwT[0][:], zwTb[:, sl],
                             start=True, stop=True)
            nc.tensor.matmul(pz[64:128, :], lwT[1][:],
                             zwTb[:, G + ch * 512:G + (ch + 1) * 512],
                             start=True, stop=True)
            # relu(x + bias) on DVE: (psum add linb2) max 0
            nc.vector.tensor_scalar(ztT2[:, sl], pz[:], linb2[:], 0.0,
                                    op0=ALU.add, op1=ALU.max)

        # ---------- hT2 [64, 2048] bf16: tanh(zt @ a1w.T + b), block-diag ----------
        hT2 = pers.tile([64, G], BF16, tag="hT2")
        for ch in range(4):
            ph = psml.tile([128, 512], F32, tag="ps")
            sl = slice(ch * 512, (ch + 1) * 512)
            nc.tensor.matmul(ph[0:64, :], a1wB[:], ztT2[:, sl],
                             start=True, stop=True)
            nc.scalar.activation(hT2[:, sl], ph[0:64, :], AF.Tanh, bias=a1b2[:])

        # ---------- scores -> eT2 [2, 2048] bf16, S2 [2,1] f32 ----------
        eT2 = pers.tile([2, G], BF16, tag="eT2")
        Sacc = pers.tile([2, 4], F32, tag="Sacc")
        for ch in range(4):
            ps_ = psml.tile([128, 512], F32, tag="ps")
            sl = slice(ch * 512, (ch + 1) * 512)
            nc.tensor.matmul(ps_[0:2, :], a2wB[:], hT2[:, sl],
                             start=True, stop=True)
            nc.scalar.activation(eT2[:, sl], ps_[0:2, :], AF.Exp,
                                 accum_out=Sacc[:, ch:ch + 1])
        S2 = pers.tile([2, 1], F32, tag="S2")
        nc.vector.reduce_sum(S2[:], Sacc[:], axis=mybir.AxisListType.X)

        # ---------- betaT2 = 1/(e - S)  (= -1/(S - e)) ----------
        bT2a = pers.tile([2, G], F32, tag="bT2a")
        nc.vector.tensor_scalar(bT2a[:], eT2[:], S2[:], None, op0=ALU.subtract)
        bT2 = pers.tile([2, G], BF16, tag="bT2")
        with nc.allow_low_precision(reason="beta in bf16 for PE outer-product"):
            nc.vector.reciprocal(bT2[:], bT2a[:])

        # ---------- ztw = zt * e_bcast (ttr also accumulates T), then pooled ----------
        ztwT2 = pers.tile([128, G], F32, tag="ztwT2")
        Tacc = pers.tile([128, 4], F32, tag="Tacc")
        for ch in range(4):
            sl = slice(ch * 512, (ch + 1) * 512)
            peb = psml.tile([128, 512], F32, tag="ps")
            nc.tensor.matmul(peb[:], sel2[:], eT2[:, sl], start=True, stop=True)
            nc.vector.tensor_tensor(ztwT2[:, sl], ztT2[:, sl], peb[:],
                                    op=ALU.mult)
            nc.vector.reduce_sum(Tacc[:, ch:ch + 1], ztwT2[:, sl],
                                 axis=mybir.AxisListType.X)
        T2 = pers.tile([128, 1], F32, tag="T2")
        nc.vector.reduce_sum(T2[:], Tacc[:], axis=mybir.AxisListType.X)

        # pooled = (ztw - T) * beta_bcast   (beta = -1/(S-e) so signs cancel)
        pooled2 = pers.tile([128, G], F32, tag="pooled2")
        for ch in range(4):
            sl = slice(ch * 512, (ch + 1) * 512)
            pbb = psml.tile([128, 512], F32, tag="ps")
            nc.tensor.matmul(pbb[:], sel2[:], bT2[:, sl], start=True, stop=True)
            nc.vector.scalar_tensor_tensor(
                out=pooled2[:, sl], in0=ztwT2[:, sl], scalar=T2[:], in1=pbb[:],
                op0=ALU.subtract, op1=ALU.mult)

        # V rows 64:128: group1 pooled at cols 2048:4096 (converting copy),
        # group0 via partition-shifting sbuf->sbuf DMA (bit-identical f32).
        nc.vector.tensor_copy(V[64:128, G:B], pooled2[64:128, :])
        nc.sync.dma_start(V[64:128, 0:G], _r(pooled2[0:64, :]))

        # ---------- main loop: total rows (own 512) x all 4096 cols ----------
        seacc = pers.tile([128, 16], F32, tag="seacc")
        for ic in range(4):
            usl = slice(ic * 128, (ic + 1) * 128)
            for pair in range(4):
                g = pair // 2
                U = U0 if g == 0 else U1
                pm = pbig.tile([128, 1024], F32, tag="pb")
                for half in range(2):
                    jt = pair * 2 + half
                    nc.tensor.matmul(
                        pm[:, half * 512:(half + 1) * 512],
                        U[:, usl],
                        V[:, jt * 512:(jt + 1) * 512],
                        start=True, stop=True)
                es = scr.tile([128, 1024], BF16, tag="escr")
                nc.scalar.activation(
                    es[:], pm[:], AF.Exp,
                    bias=biasS[:, g * 4 + ic:g * 4 + ic + 1],
                    accum_out=seacc[:, ic * 4 + pair:ic * 4 + pair + 1])

        seall = pers.tile([128, 4], F32, tag="seall")
        for ic in range(4):
            nc.vector.reduce_sum(seall[:, ic:ic + 1], seacc[:, ic * 4:(ic + 1) * 4],
                                 axis=mybir.AxisListType.X)
        lnall = pers.tile([128, 4], F32, tag="lnall")
        nc.scalar.activation(lnall[:], seall[:], AF.Ln)

        # ---------- diagonal: diag[i] = UOwn[:,i]·V[:,own_pos(i)] ----------
        if static_diag:
            vsl = slice(0, OWN)
        else:
            pid = nc.vector.partition_id()
            vsl = bass.ts(pid, OWN)
        prod2 = pers.tile([128, OWN], BF16, tag="prod2")
        nc.vector.tensor_tensor(prod2[:], UOwn[:].bitcast(F32),
                                V[:, vsl].bitcast(F32), op=ALU.mult)

        vall = pers.tile([128, 4], F32, tag="vall")
        for ic in range(4):
            pdg = ptin.tile([128, 1], F32, tag="pt")
            nc.tensor.matmul(pdg[:], prod2[:, ic * 128:(ic + 1) * 128], ones[:],
                             start=True, stop=False)
            nc.tensor.matmul(pdg[:], zwoTb[:, ic * 128:(ic + 1) * 128], bo_s[:],
                             start=False, stop=True)
            # v = (diag_raw + delta - 44) - ln(sumexp)
            nc.vector.scalar_tensor_tensor(
                out=vall[:, ic:ic + 1], in0=pdg[:], scalar=-SHIFT,
                in1=lnall[:, ic:ic + 1], op0=ALU.add, op1=ALU.subtract)

        nc.sync.dma_start(v_d[:], vall[:])

    nc.compile()
    return nc


_built = None


def _get_program():
    global _built
    if _built is None:
        _built = _build_program()
    return _built


def make_in_maps(inputs):
    import ml_dtypes
    BF = ml_dtypes.bfloat16
    f = lambda x: np.ascontiguousarray(np.asarray(x, dtype=np.float32))
    bf = lambda x: np.ascontiguousarray(np.asarray(x, np.float32).astype(BF))

    zw = np.concatenate([f(inputs['zw_0']), f(inputs['zw_1'])], axis=0)
    zwT = np.ascontiguousarray(zw.T)
    b0 = f(inputs['Ww0_b']) + f(inputs['Wk_b'])
    b1 = f(inputs['Ww1_b']) + f(inputs['Wk_b'])

    a1wB = np.zeros((128, 64), np.float32)
    a1wB[0:64, 0:32] = f(inputs['a0_1w']).T
    a1wB[64:128, 32:64] = f(inputs['a1_1w']).T
    a2wB = np.zeros((64, 2), np.float32)
    a2wB[0:32, 0:1] = f(inputs['a0_2w']).T
    a2wB[32:64, 1:2] = f(inputs['a1_2w']).T
    sel2 = np.zeros((2, 128), np.float32)
    sel2[0, 0:64] = 1.0
    sel2[1, 64:128] = 1.0
    linb2 = np.concatenate([f(inputs['lin0_b']), f(inputs['lin1_b'])])
    a1b2 = np.concatenate([f(inputs['a0_1b']), f(inputs['a1_1b'])])
    wk = f(inputs['Wk_w'])
    uw0 = np.hstack([wk, f(inputs['Ww0_w'])])   # [128,128]
    uw1 = np.hstack([wk, f(inputs['Ww1_w'])])

    base = {
        'zwTb': bf(zwT),
        'cT': np.ascontiguousarray(f(inputs['c']).T),
        'UW0': uw0,
        'UW1': uw1,
        'lwT0': bf(f(inputs['lin0_w']).T),
        'lwT1': bf(f(inputs['lin1_w']).T),
        'a1wB': bf(a1wB),
        'a2wB': bf(a2wB),
        'b0': bf(b0.reshape(128, 1)),
        'b1': bf(b1.reshape(128, 1)),
        'sel2': bf(sel2),
        'ones': bf(np.ones((128, 1), np.float32)),
        'linb2': linb2.reshape(128, 1),
        'a1b2': a1b2.reshape(64, 1),
    }
    in_maps = []
    for cid in range(N_CORES):
        g = cid // 4
        m = dict(base)
        zo = np.ascontiguousarray(zwT[:, cid * OWN:(cid + 1) * OWN])
        m['zwoT'] = zo
        m['zwoTb'] = bf(zo)
        m['UWo'] = uw0 if g == 0 else uw1
        m['b_own'] = bf((b0 if g == 0 else b1).reshape(128, 1))
        in_maps.append(m)
    return in_maps


def kernel(**inputs):
    nc = _get_program()
    in_maps = make_in_maps(inputs)
    res = run_bass_kernel_spmd(nc, in_maps, list(range(N_CORES)))
    tot = 0.0
    for r in res.results:
        tot += np.asarray(r['v'], dtype=np.float64).sum()
    return np.array(-(tot / B), dtype=np.float32)



# revision 19
# speedup vs baseline: 1.0083x; 1.0083x over previous
"""Trainium2 Bass kernel for nn_CPCModel (CPC-style NCE loss).

Strategy (8 NeuronCores, full inputs on every core, no collectives):

The reference's leave-one-out softmax pooling collapses algebraically:
    pooled[i] = (T - e_i * zt_i) / (S - e_i),  e = exp(s), S = sum(e), T = sum(e_j zt_j)
so the [B,B] pooling matrix is never materialized.  The loss needs only
    nce = -mean_i( total[i,i] - logsumexp_j total[i,j] )
with  total[i, j in group g] = Azw_g[i]·pooled_g[j] + Czw[i]·c[j] + delta_g[i]
where Azw_g = zw @ Ww_g, Czw = zw @ Wk_w, delta_g = zw @ (Ww_g_b + Wk_b).

Each core redundantly computes the cheap pooling prep for all 4096 rows
and its own 512 rows of the [4096,4096] total matrix + row-wise
sum(exp(total - 44)) on the scalar engine with accum_out.  The main-loop
matmuls split the 128-contraction into a c-half and a pooled-half that
accumulate in PSUM; both halves stay K=128 (fp32r silently dies at
runtime with K=64) by zero-padding the unused 64 partitions of each rhs:
  cV    = [cT ; 0]              (c-half; U1's Azw rows hit the zeros)
  P2own = [0 ; pooled0|pooled1] (pooled-half; lhsT UA_g = [0 ; Azw_g])
P2own doubles as the diagonal's own-pooled source (one dynamic slice
works for every core).  Raw per-row exp sums and raw diagonal values
ship to the host, which finishes with log() in float64.

Engine balance: prep relu/combine/reciprocal on DVE, tanh/exp on the
scalar engine (one activation-table set for the whole kernel: relu,
tanh, exp, copy all live in exp_and_others), zero-fills on GPSIMD,
(de)broadcasts on the PE via tiny selector matmuls.  Inputs arrive in 4
batched DMAs.
"""

import numpy as np

import concourse.bacc as bacc
import concourse.bass as bass
import concourse.mybir as mybir
import concourse.tile as tile
from concourse.bass_utils import run_bass_kernel_spmd

N_CORES = 8
B = 4096
OWN = B // N_CORES            # 512 rows of `total` per core
G = 2048                      # group size
F32 = mybir.dt.float32
F32R = mybir.dt.float32r
BF16 = mybir.dt.bfloat16
AF = mybir.ActivationFunctionType
ALU = mybir.AluOpType
SHIFT = 44.0

# bf16 bundle column offsets
ZW = 0            # zwTb [128, 4096]
ZWO = 4096        # zwoTb [128, 512]
LW0 = 4608        # lin0_w.T [128, 64]
LW1 = 4672        # lin1_w.T [128, 64]
A1W = 4736        # blockdiag(a0_1w.T, a1_1w.T) [128, 64]
A2W = 4800        # blockdiag(a0_2w.T, a1_2w.T) [64, 2]
SEL = 4802        # sel2 [2, 128]
BSTK = 4930       # [b0 | b1] [128, 2]
BOWN = 4932       # b_{group(core)} [128, 1]
BONE = 4933       # ones [128, 1]
NBF = 4934

# f32r bundle column offsets
ZWOF = 0          # zwoT [128, 512]
CU0 = 512         # hstack(Wk_w, Ww0_w) [128, 128]
CU1 = 640         # hstack(Wk_w, Ww1_w)
CUWO = 768        # per-core hstack(Wk_w, Ww_g) [128, 128] (diag)
NR = 896
# f32 bundle column offsets
LINB = 0          # [lin0_b ; lin1_b] [128, 1]
A1B = 1           # [a0_1b ; a1_1b] [64, 1]
A2B = 2           # [a0_2b ; a1_2b] [2, 1]
NF32 = 3


def _build_program(static_diag=False, upto='full'):
    nc = bacc.Bacc(
        "TRN2",
        target_bir_lowering=False,
        debug=False,
        num_devices=N_CORES,
    )

    bigbf_d = nc.dram_tensor("bigbf", [128, NBF], BF16, kind="ExternalInput").ap()
    bigr_d = nc.dram_tensor("bigr", [128, NR], F32R, kind="ExternalInput").ap()
    bigf_d = nc.dram_tensor("bigf", [128, NF32], F32, kind="ExternalInput").ap()
    cT_d = nc.dram_tensor("cT", [64, B], F32R, kind="ExternalInput").ap()
    vout_d = nc.dram_tensor("vout", [128, 20], F32, kind="ExternalOutput").ap()

    from contextlib import ExitStack
    with tile.TileContext(nc) as tc, ExitStack() as ctx:
        pers = ctx.enter_context(tc.tile_pool(name="pers", bufs=1))
        scr = ctx.enter_context(tc.tile_pool(name="scr", bufs=2))

        bb = pers.tile([128, NBF], BF16, tag="bb", name="bb")
        nc.sync.dma_start(bb[:], bigbf_d[:])
        br = pers.tile([128, NR], F32R, tag="br", name="br")
        nc.sync.dma_start(br[:], bigr_d[:])
        bf = pers.tile([128, NF32], F32, tag="bf", name="bf")
        nc.sync.dma_start(bf[:], bigf_d[:])
        # V2 = [cT ; pooled0|pooled1]: the loop rhs AND the diagonal source
        V2 = pers.tile([128, B], F32R, tag="V2", name="V2")
        nc.sync.dma_start(V2[0:64, :], cT_d[:])

        zwoT = br[:, ZWOF:ZWOF + 512]
        sel2 = bb[0:2, SEL:SEL + 128]
        linb2 = bf[:, LINB:LINB + 1]
        a1b2 = bf[0:64, A1B:A1B + 1]
        a2b2 = bf[0:2, A2B:A2B + 1]

        ztT2 = pers.tile([128, G], BF16, tag="ztT2")
        hT2 = pers.tile([64, G], BF16, tag="hT2")
        eT2 = pers.tile([2, G], BF16, tag="eT2")
        d2 = pers.tile([2, G], BF16, tag="d2")
        bT2 = pers.tile([2, G], BF16, tag="bT2")
        ztwT2 = pers.tile([128, G], F32, tag="ztwT2")
        pooled2 = pers.tile([128, G], F32R, tag="pooled2")
        Tacc = pers.tile([128, 4], F32, tag="Tacc")
        T2 = pers.tile([128, 1], F32, tag="T2")
        S2 = pers.tile([2, 1], F32, tag="S2")
        biasS = pers.tile([128, 8], F32, tag="biasS")
        U0 = pers.tile([128, OWN], F32R, tag="U0")
        U1 = pers.tile([128, OWN], F32R, tag="U1")
        UOwnS = pers.tile([128, OWN], F32, tag="UOwnS")
        vout = pers.tile([128, 20], F32, tag="vout")

        with tc.tile_pool(name="psA", bufs=3, space="PSUM") as psA, \
             tc.tile_pool(name="psB", bufs=1, space="PSUM") as psB, \
             tc.tile_pool(name="psS", bufs=1, space="PSUM") as psS:
            # ---- U tiles (own rows), biasS = zw_own·b_g - 44 ----
            for U, uoff in [(U0, CU0), (U1, CU1)]:
                pu = psA.tile([128, 512], F32, tag="ps")
                nc.tensor.matmul(pu[:], br[:, uoff:uoff + 128],
                                 zwoT, start=True, stop=True)
                nc.vector.tensor_copy(U[:], pu[:])

            pbias = psB.tile([128, 8], F32, tag="pb")
            for ic in range(4):
                nc.tensor.matmul(pbias[:, ic * 2:ic * 2 + 2],
                                 bb[:, ZWO + ic * 128:ZWO + (ic + 1) * 128],
                                 bb[:, BSTK:BSTK + 2], start=True, stop=True)
            nc.vector.tensor_scalar(biasS[:], pbias[:], -SHIFT, None,
                                    op0=ALU.add)

            # ---- zt -> h -> s chain (stacked groups, 4 chunks of 512) ----
            sP = psS.tile([2, G], F32, tag="sp")
            for ch in range(4):
                sl = slice(ch * 512, (ch + 1) * 512)
                pz = psA.tile([128, 512], F32, tag="ps")
                nc.tensor.matmul(pz[0:64, :], bb[:, LW0:LW0 + 64],
                                 bb[:, ZW + ch * 512:ZW + (ch + 1) * 512],
                                 start=True, stop=True)
                nc.tensor.matmul(pz[64:128, :], bb[:, LW1:LW1 + 64],
                                 bb[:, ZW + G + ch * 512:ZW + G + (ch + 1) * 512],
                                 start=True, stop=True)
                # relu(x + bias) on DVE (GPSIMD can't read PSUM)
                nc.vector.tensor_scalar(ztT2[:, sl], pz[:], linb2, 0.0,
                                        op0=ALU.add, op1=ALU.max)
                ph = psA.tile([128, 512], F32, tag="ps")
                nc.tensor.matmul(ph[0:64, :], bb[:, A1W:A1W + 64], ztT2[:, sl],
                                 start=True, stop=True)
                nc.scalar.activation(hT2[:, sl], ph[0:64, :], AF.Tanh,
                                     bias=a1b2)
                nc.tensor.matmul(sP[:, sl], bb[0:64, A2W:A2W + 2], hT2[:, sl],
                                 start=True, stop=True)

            # ---- e = exp(s + a2b), S = rowsum(e) : one wide activation ----
            nc.scalar.activation(eT2[:], sP[:], AF.Exp, bias=a2b2,
                                 accum_out=S2[:])

            # ---- d = e - S (DVE 4x), beta = 1/d chunked; ztw, T ----
            nc.vector.tensor_scalar(d2[:], eT2[:], S2[:], None,
                                    op0=ALU.subtract)
            for ch in range(4):
                sl = slice(ch * 512, (ch + 1) * 512)
                peb = psA.tile([128, 512], F32, tag="ps")
                nc.tensor.matmul(peb[:], sel2, eT2[:, sl], start=True, stop=True)
                nc.vector.tensor_tensor(ztwT2[:, sl], ztT2[:, sl], peb[:],
                                        op=ALU.mult)
                nc.vector.reduce_sum(Tacc[:, ch:ch + 1], ztwT2[:, sl],
                                     axis=mybir.AxisListType.X)
                with nc.allow_low_precision(reason="beta in bf16 for PE bcast"):
                    nc.vector.reciprocal(bT2[:, sl], d2[:, sl])
            nc.vector.reduce_sum(T2[:], Tacc[:], axis=mybir.AxisListType.X)

            # ---- pooled = (ztw - T) * beta_bcast ----
            for ch in range(4):
                sl = slice(ch * 512, (ch + 1) * 512)
                pbb = psA.tile([128, 512], F32, tag="ps")
                nc.tensor.matmul(pbb[:], sel2, bT2[:, sl], start=True, stop=True)
                nc.vector.scalar_tensor_tensor(
                    out=pooled2[:, sl], in0=ztwT2[:, sl], scalar=T2[:],
                    in1=pbb[:], op0=ALU.subtract, op1=ALU.mult)

            # ---- UOwn for the diagonal ----
            puo = psA.tile([128, 512], F32, tag="ps")
            nc.tensor.matmul(puo[:], br[:, CUWO:CUWO + 128],
                             zwoT, start=True, stop=True)
            nc.vector.tensor_copy(UOwnS[:], puo[:])

        if upto == 'prep':
            nc.vector.tensor_copy(vout[:], pooled2[:, 0:20].bitcast(F32))
            nc.sync.dma_start(vout_d[:], vout[:])

        # V2 rows 64:128: cols 0:2048 = pooled0, cols 2048:4096 = pooled1
        nc.sync.dma_start(V2[64:128, 0:G], pooled2[0:64, :])
        nc.sync.dma_start(V2[64:128, G:B], pooled2[64:128, :])

        # ---- main loop: 16 chunks of [128 own rows, 1024 group cols] ----
        with tc.tile_pool(name="pbig", bufs=4, space="PSUM") as pbig:
          if upto != 'prep':
            for g in range(2):
                Ug = U0 if g == 0 else U1
                for ic in range(4):
                    usl = slice(ic * 128, (ic + 1) * 128)
                    for h in range(2):
                        pm = pbig.tile([128, 1024], F32, tag="pb")
                        for q in range(2):
                            qs = slice(q * 512, (q + 1) * 512)
                            c0 = g * G + h * 1024 + q * 512
                            nc.tensor.matmul(pm[:, qs], Ug[:, usl],
                                             V2[:, c0:c0 + 512],
                                             start=True, stop=True)
                        es = scr.tile([128, 1024], BF16, tag="es")
                        cc = g * 8 + ic * 2 + h
                        nc.scalar.activation(es[:], pm[:], AF.Exp,
                                             bias=biasS[:, 2 * ic + g:
                                                        2 * ic + g + 1],
                                             accum_out=vout[:, cc:cc + 1])

        if upto == 'loop':
            nc.sync.dma_start(vout_d[:], vout[:])

        # ---- diagonal: diag[i] = UOwn[:,i]·[c;pooled][:,own(i)] + delta ----
        with tc.tile_pool(name="ptail", bufs=1, space="PSUM") as pt:
          if upto == 'full':
            if static_diag:
                csl = slice(0, OWN)
            else:
                pid = nc.vector.partition_id()
                csl = bass.ts(pid, OWN)
            prod = pers.tile([128, OWN], BF16, tag="prod")
            nc.vector.tensor_tensor(prod[0:64, :], UOwnS[0:64, :],
                                    V2[0:64, csl].bitcast(F32), op=ALU.mult)
            nc.vector.tensor_tensor(prod[64:128, :], UOwnS[64:128, :],
                                    V2[64:128, csl].bitcast(F32),
                                    op=ALU.mult)
            pdg = pt.tile([128, 4], F32, tag="pt")
            for ic in range(4):
                nc.tensor.matmul(pdg[:, ic:ic + 1],
                                 prod[:, ic * 128:(ic + 1) * 128],
                                 bb[:, BONE:BONE + 1], start=True, stop=False)
                nc.tensor.matmul(pdg[:, ic:ic + 1],
                                 bb[:, ZWO + ic * 128:ZWO + (ic + 1) * 128],
                                 bb[:, BOWN:BOWN + 1], start=False, stop=True)
            nc.vector.tensor_copy(vout[:, 16:20], pdg[:])
            nc.sync.dma_start(vout_d[:], vout[:])

    nc.compile()
    return nc


_built = None


def _get_program():
    global _built
    if _built is None:
        _built = _build_program()
    return _built


def make_in_maps(inputs):
    import ml_dtypes
    BF = ml_dtypes.bfloat16
    f = lambda x: np.asarray(x, dtype=np.float32)

    zw = np.concatenate([f(inputs['zw_0']), f(inputs['zw_1'])], axis=0)
    zwT = np.ascontiguousarray(zw.T)                  # [128, 4096]
    b0 = f(inputs['Ww0_b']) + f(inputs['Wk_b'])
    b1 = f(inputs['Ww1_b']) + f(inputs['Wk_b'])
    wk = f(inputs['Wk_w'])
    uw0 = np.hstack([wk, f(inputs['Ww0_w'])])          # [Czw ; Azw0]
    uw1 = np.hstack([wk, f(inputs['Ww1_w'])])          # [Czw ; Azw1]

    bigbf = np.zeros((128, NBF), np.float32)
    bigbf[:, ZW:ZW + B] = zwT
    bigbf[:, LW0:LW0 + 64] = f(inputs['lin0_w']).T
    bigbf[:, LW1:LW1 + 64] = f(inputs['lin1_w']).T
    bigbf[0:64, A1W:A1W + 32] = f(inputs['a0_1w']).T
    bigbf[64:128, A1W + 32:A1W + 64] = f(inputs['a1_1w']).T
    bigbf[0:32, A2W:A2W + 1] = f(inputs['a0_2w']).T
    bigbf[32:64, A2W + 1:A2W + 2] = f(inputs['a1_2w']).T
    bigbf[0, SEL:SEL + 64] = 1.0
    bigbf[1, SEL + 64:SEL + 128] = 1.0
    bigbf[:, BSTK] = b0
    bigbf[:, BSTK + 1] = b1
    bigbf[:, BONE] = 1.0

    bigr = np.zeros((128, NR), np.float32)
    bigr[:, CU0:CU0 + 128] = uw0
    bigr[:, CU1:CU1 + 128] = uw1
    bigf = np.zeros((128, NF32), np.float32)
    bigf[:, LINB] = np.concatenate([f(inputs['lin0_b']), f(inputs['lin1_b'])])
    bigf[0:64, A1B] = np.concatenate([f(inputs['a0_1b']), f(inputs['a1_1b'])])
    bigf[0:2, A2B] = np.concatenate([f(inputs['a0_2b']), f(inputs['a1_2b'])])

    cT = np.ascontiguousarray(f(inputs['c']).T)        # [64, 4096]

    in_maps = []
    for cid in range(N_CORES):
        g = cid // 4
        mbf = bigbf.copy()
        mbf[:, ZWO:ZWO + OWN] = zwT[:, cid * OWN:(cid + 1) * OWN]
        mbf[:, BOWN] = b0 if g == 0 else b1
        mr = bigr.copy()
        mr[:, ZWOF:ZWOF + OWN] = zwT[:, cid * OWN:(cid + 1) * OWN]
        mr[:, CUWO:CUWO + 128] = uw0 if g == 0 else uw1
        in_maps.append({
            'bigbf': np.ascontiguousarray(mbf.astype(BF)),
            'bigr': np.ascontiguousarray(mr),
            'bigf': bigf,
            'cT': cT,
        })
    return in_maps


def kernel(**inputs):
    nc = _get_program()
    in_maps = make_in_maps(inputs)
    res = run_bass_kernel_spmd(nc, in_maps, list(range(N_CORES)))
    tot = 0.0
    for r in res.results:
        v = np.asarray(r['vout'], dtype=np.float64)
        sec = v[:, 0:16].reshape(128, 2, 4, 2)   # [i, g, ic, h]
        se = sec.sum(axis=(1, 3))                # [128, 4]
        dg = v[:, 16:20]
        tot += np.sum(dg - SHIFT - np.log(se))
    return np.array(-(tot / B), dtype=np.float32)


# revision 21
# speedup vs baseline: 1.0741x; 1.0652x over previous
"""Trainium2 Bass kernel for nn_CPCModel (CPC-style NCE loss).

Strategy (8 NeuronCores, full inputs on every core, no collectives):

The reference's leave-one-out softmax pooling collapses algebraically:
    pooled[i] = (T - e_i * zt_i) / (S - e_i),  e = exp(s), S = sum(e), T = sum(e_j zt_j)
so the [B,B] pooling matrix is never materialized.  The loss needs only
    nce = -mean_i( total[i,i] - logsumexp_j total[i,j] )
with  total[i, j in group g] = Azw_g[i]·pooled_g[j] + Czw[i]·c[j] + delta_g[i]
where Azw_g = zw @ Ww_g, Czw = zw @ Wk_w, delta_g = zw @ (Ww_g_b + Wk_b).

Each core redundantly computes the cheap pooling prep for all 4096 rows
(both groups stacked on the 128 partitions) and its own 512 rows of the
[4096,4096] total matrix + row-wise sum(exp(total - 44)) via 8
[128,2048] exp chunks on the scalar engine with accum_out.  The loop
rhs V2 = [cT ; pooled0|pooled1] is assembled by three DMAs (no compute)
and doubles as the diagonal's source through one partition_id-dynamic
column slice.  Raw per-row exp sums and raw diagonal values ship to the
host, which finishes with log() in float64.

Scheduling: weights arrive in a small first DMA and zw^T in two
chunk-major halves so the zt->h->s chain starts ~4us in; exp-of-s is
chunked so the fused ztw/T tensor_tensor_reduce starts before the last
tanh; U/bias matmuls and paced PE probes keep the tensor engine out of
its low p-state before the main loop.  fp32r matmuls keep K=128
throughout (K=64 fp32r dies at runtime).  One activation-table set
(exp_and_others: relu/tanh/exp/copy) serves the whole kernel; the final
log runs on the host.
"""

import numpy as np

import concourse.bacc as bacc
import concourse.bass as bass
import concourse.mybir as mybir
import concourse.tile as tile
from concourse.bass_utils import run_bass_kernel_spmd

N_CORES = 8
B = 4096
OWN = B // N_CORES            # 512 rows of `total` per core
G = 2048                      # group size
F32 = mybir.dt.float32
F32R = mybir.dt.float32r
BF16 = mybir.dt.bfloat16
AF = mybir.ActivationFunctionType
ALU = mybir.AluOpType
SHIFT = 44.0

# bf16 weights bundle column offsets
ZWO = 0           # zwoTb [128, 512]
LW0 = 512         # lin0_w.T [128, 64]
LW1 = 576         # lin1_w.T [128, 64]
A1W = 640         # blockdiag(a0_1w.T, a1_1w.T) [128, 64]
A2W = 704         # blockdiag(a0_2w.T, a1_2w.T) [64, 2]
SEL = 706         # sel2 [2, 128]
BSTK = 834        # [b0 | b1] [128, 2]
BOWN = 836        # b_{group(core)} [128, 1]
BONE = 837        # ones [128, 1]
WN = 838

# f32r bundle column offsets
ZWOF = 0          # zwoT [128, 512]
CU0 = 512         # hstack(Wk_w, Ww0_w) [128, 128]
CU1 = 640         # hstack(Wk_w, Ww1_w)
CUWO = 768        # per-core hstack(Wk_w, Ww_g) [128, 128] (diag)
NR = 896
# f32 bundle column offsets
LINB = 0          # [lin0_b ; lin1_b] [128, 1]
A1B = 1           # [a0_1b ; a1_1b] [64, 1]
A2B = 2           # [a0_2b ; a1_2b] [2, 1]
NF32 = 3


def _build_program(static_diag=False):
    nc = bacc.Bacc(
        "TRN2",
        target_bir_lowering=False,
        debug=False,
        num_devices=N_CORES,
    )

    bigw_d = nc.dram_tensor("bigw", [128, WN], BF16, kind="ExternalInput").ap()
    zwc_d = nc.dram_tensor("zwc", [128, B], BF16, kind="ExternalInput").ap()
    bigr_d = nc.dram_tensor("bigr", [128, NR], F32R, kind="ExternalInput").ap()
    bigf_d = nc.dram_tensor("bigf", [128, NF32], F32, kind="ExternalInput").ap()
    cT_d = nc.dram_tensor("cT", [64, B], F32R, kind="ExternalInput").ap()
    vout_d = nc.dram_tensor("vout", [128, 12], F32, kind="ExternalOutput").ap()

    from contextlib import ExitStack
    with tile.TileContext(nc) as tc, ExitStack() as ctx:
        pers = ctx.enter_context(tc.tile_pool(name="pers", bufs=1))
        scr = ctx.enter_context(tc.tile_pool(name="scr", bufs=2))

        # DMA order = need order: weights, zw halves, f32r/f32 bundles, cT
        bw = pers.tile([128, WN], BF16, tag="bw", name="bw")
        nc.sync.dma_start(bw[:], bigw_d[:])
        zwc = pers.tile([128, B], BF16, tag="zwc", name="zwc")
        nc.sync.dma_start(zwc[:, 0:G], zwc_d[:, 0:G])
        nc.sync.dma_start(zwc[:, G:B], zwc_d[:, G:B])
        br = pers.tile([128, NR], F32R, tag="br", name="br")
        nc.sync.dma_start(br[:], bigr_d[:])
        bf = pers.tile([128, NF32], F32, tag="bf", name="bf")
        nc.sync.dma_start(bf[:], bigf_d[:])
        # V2 = [cT ; pooled0|pooled1]: loop rhs AND diagonal source
        V2 = pers.tile([128, B], F32R, tag="V2", name="V2")
        nc.sync.dma_start(V2[0:64, :], cT_d[:])

        zwoT = br[:, ZWOF:ZWOF + 512]
        sel2 = bw[0:2, SEL:SEL + 128]
        linb2 = bf[:, LINB:LINB + 1]
        a1b2 = bf[0:64, A1B:A1B + 1]
        a2b2 = bf[0:2, A2B:A2B + 1]

        ztT2 = pers.tile([128, G], BF16, tag="ztT2")
        hT2 = pers.tile([64, G], BF16, tag="hT2")
        eT2 = pers.tile([2, G], BF16, tag="eT2")
        d2 = pers.tile([2, G], BF16, tag="d2")
        bT2 = pers.tile([2, G], BF16, tag="bT2")
        ztwT2 = pers.tile([128, G], F32, tag="ztwT2")
        pooled2 = pers.tile([128, G], F32R, tag="pooled2")
        Sacc = pers.tile([2, 4], F32, tag="Sacc")
        Tacc = pers.tile([128, 4], F32, tag="Tacc")
        T2 = pers.tile([128, 1], F32, tag="T2")
        S2 = pers.tile([2, 1], F32, tag="S2")
        biasS = pers.tile([128, 8], F32, tag="biasS")
        U0 = pers.tile([128, OWN], F32R, tag="U0")
        U1 = pers.tile([128, OWN], F32R, tag="U1")
        UOwnS = pers.tile([128, OWN], F32, tag="UOwnS")
        vout = pers.tile([128, 12], F32, tag="vout")

        with tc.tile_pool(name="psA", bufs=3, space="PSUM") as psA, \
             tc.tile_pool(name="psB", bufs=1, space="PSUM") as psB, \
             tc.tile_pool(name="psU", bufs=3, space="PSUM") as psU:
            # ---- zt -> h -> s -> e chain (stacked groups, 4 x 512 cols) ----
            for ch in range(4):
                sl = slice(ch * 512, (ch + 1) * 512)
                pz = psA.tile([128, 512], F32, tag="ps")
                nc.tensor.matmul(pz[0:64, :], bw[:, LW0:LW0 + 64],
                                 zwc[:, ch * 1024:ch * 1024 + 512],
                                 start=True, stop=True)
                nc.tensor.matmul(pz[64:128, :], bw[:, LW1:LW1 + 64],
                                 zwc[:, ch * 1024 + 512:(ch + 1) * 1024],
                                 start=True, stop=True)
                # relu(x + bias) on DVE (GPSIMD can't read PSUM)
                nc.vector.tensor_scalar(ztT2[:, sl], pz[:], linb2, 0.0,
                                        op0=ALU.add, op1=ALU.max)
                ph = psA.tile([128, 512], F32, tag="ps")
                nc.tensor.matmul(ph[0:64, :], bw[:, A1W:A1W + 64], ztT2[:, sl],
                                 start=True, stop=True)
                nc.scalar.activation(hT2[:, sl], ph[0:64, :], AF.Tanh,
                                     bias=a1b2)
                ps_ = psA.tile([128, 512], F32, tag="ps")
                nc.tensor.matmul(ps_[0:2, :], bw[0:64, A2W:A2W + 2], hT2[:, sl],
                                 start=True, stop=True)
                nc.scalar.activation(eT2[:, sl], ps_[0:2, :], AF.Exp,
                                     bias=a2b2, accum_out=Sacc[:, ch:ch + 1])

            # ---- e-broadcast + fused ztw/T (DVE order: after all relus) ----
            for ch in range(4):
                sl = slice(ch * 512, (ch + 1) * 512)
                peb = psU.tile([128, 512], F32, tag="pu")
                nc.tensor.matmul(peb[:], sel2, eT2[:, sl], start=True, stop=True)
                nc.vector.tensor_tensor(ztwT2[:, sl], ztT2[:, sl], peb[:],
                                        op=ALU.mult)
                nc.vector.reduce_sum(Tacc[:, ch:ch + 1], ztwT2[:, sl],
                                     axis=mybir.AxisListType.X)

            # ---- beta = 1/(e - S) ----
            nc.vector.reduce_sum(S2[:], Sacc[:], axis=mybir.AxisListType.X)
            nc.vector.tensor_scalar(d2[:], eT2[:], S2[:], None,
                                    op0=ALU.subtract)
            for ch in range(4):
                sl = slice(ch * 512, (ch + 1) * 512)
                with nc.allow_low_precision(reason="beta in bf16 for PE bcast"):
                    nc.vector.reciprocal(bT2[:, sl], d2[:, sl])
            nc.vector.reduce_sum(T2[:], Tacc[:], axis=mybir.AxisListType.X)

            # ---- U tiles + delta biases (PE slack while beta computes) ----
            for U, uoff in [(U0, CU0), (U1, CU1)]:
                pu = psU.tile([128, 512], F32, tag="pu")
                nc.tensor.matmul(pu[:], br[:, uoff:uoff + 128],
                                 zwoT, start=True, stop=True)
                nc.vector.tensor_copy(U[:], pu[:])
            puo = psU.tile([128, 512], F32, tag="pu")
            nc.tensor.matmul(puo[:], br[:, CUWO:CUWO + 128],
                             zwoT, start=True, stop=True)
            nc.vector.tensor_copy(UOwnS[:], puo[:])
            pbias = psB.tile([128, 8], F32, tag="pb")
            for ic in range(4):
                nc.tensor.matmul(pbias[:, ic * 2:ic * 2 + 2],
                                 bw[:, ZWO + ic * 128:ZWO + (ic + 1) * 128],
                                 bw[:, BSTK:BSTK + 2], start=True, stop=True)
            nc.vector.tensor_scalar(biasS[:], pbias[:], -SHIFT, None,
                                    op0=ALU.add)

            # ---- pooled = (ztw - T) * beta_bcast ----
            for ch in range(4):
                sl = slice(ch * 512, (ch + 1) * 512)
                pbb = psA.tile([128, 512], F32, tag="ps")
                nc.tensor.matmul(pbb[:], sel2, bT2[:, sl], start=True, stop=True)
                nc.vector.scalar_tensor_tensor(
                    out=pooled2[:, sl], in0=ztwT2[:, sl], scalar=T2[:],
                    in1=pbb[:], op0=ALU.subtract, op1=ALU.mult)
                # paced junk matmul: keeps the PE out of its low p-state
                # between the prep matmuls and the main loop
                pw = psU.tile([128, 512], F32, tag="pu")
                nc.tensor.matmul(pw[:], U0[:, 0:128],
                                 pooled2[:, sl], start=True, stop=True)

        # V2 rows 64:128: cols 0:2048 = pooled0, cols 2048:4096 = pooled1
        nc.sync.dma_start(V2[64:128, 0:G], pooled2[0:64, :])
        nc.sync.dma_start(V2[64:128, G:B], pooled2[64:128, :])

        # ---- main loop: 8 chunks of [128 own rows, 2048 group cols] ----
        with tc.tile_pool(name="pbig", bufs=2, space="PSUM") as pbig:
            for g in range(2):
                Ug = U0 if g == 0 else U1
                for ic in range(4):
                    usl = slice(ic * 128, (ic + 1) * 128)
                    pm = pbig.tile([128, G], F32, tag="pb")
                    for q in range(4):
                        qs = slice(q * 512, (q + 1) * 512)
                        nc.tensor.matmul(pm[:, qs], Ug[:, usl],
                                         V2[:, g * G + q * 512:
                                            g * G + (q + 1) * 512],
                                         start=True, stop=True)
                    es = scr.tile([128, G], BF16, tag="es")
                    cc = g * 4 + ic
                    nc.scalar.activation(es[:], pm[:], AF.Exp,
                                         bias=biasS[:, 2 * ic + g:
                                                    2 * ic + g + 1],
                                         accum_out=vout[:, cc:cc + 1])

        # ---- diagonal: diag[i] = UOwn[:,i]·V2[:,own(i)] + delta ----
        with tc.tile_pool(name="ptail", bufs=1, space="PSUM") as pt:
            if static_diag:
                csl = slice(0, OWN)
            else:
                pid = nc.vector.partition_id()
                csl = bass.ts(pid, OWN)
            prod = pers.tile([128, OWN], BF16, tag="prod")
            nc.vector.tensor_tensor(prod[0:64, :], UOwnS[0:64, :],
                                    V2[0:64, csl].bitcast(F32), op=ALU.mult)
            nc.vector.tensor_tensor(prod[64:128, :], UOwnS[64:128, :],
                                    V2[64:128, csl].bitcast(F32),
                                    op=ALU.mult)
            pdg = pt.tile([128, 4], F32, tag="pt")
            for ic in range(4):
                nc.tensor.matmul(pdg[:, ic:ic + 1],
                                 prod[:, ic * 128:(ic + 1) * 128],
                                 bw[:, BONE:BONE + 1], start=True, stop=False)
                nc.tensor.matmul(pdg[:, ic:ic + 1],
                                 bw[:, ZWO + ic * 128:ZWO + (ic + 1) * 128],
                                 bw[:, BOWN:BOWN + 1], start=False, stop=True)
            nc.vector.tensor_copy(vout[:, 8:12], pdg[:])
            nc.sync.dma_start(vout_d[:], vout[:])

    nc.compile()
    return nc


_built = None


def _get_program():
    global _built
    if _built is None:
        _built = _build_program()
    return _built


def make_in_maps(inputs):
    import ml_dtypes
    BF = ml_dtypes.bfloat16
    f = lambda x: np.asarray(x, dtype=np.float32)

    zw = np.concatenate([f(inputs['zw_0']), f(inputs['zw_1'])], axis=0)
    zwT = np.ascontiguousarray(zw.T)                  # [128, 4096]
    # chunk-major layout: block ch = [group0 cols ch*512.. | group1 cols ...]
    zwc = np.empty_like(zwT)
    for ch in range(4):
        zwc[:, ch * 1024:ch * 1024 + 512] = zwT[:, ch * 512:(ch + 1) * 512]
        zwc[:, ch * 1024 + 512:(ch + 1) * 1024] = \
            zwT[:, G + ch * 512:G + (ch + 1) * 512]
    b0 = f(inputs['Ww0_b']) + f(inputs['Wk_b'])
    b1 = f(inputs['Ww1_b']) + f(inputs['Wk_b'])
    wk = f(inputs['Wk_w'])
    uw0 = np.hstack([wk, f(inputs['Ww0_w'])])          # [Czw ; Azw0]
    uw1 = np.hstack([wk, f(inputs['Ww1_w'])])          # [Czw ; Azw1]

    bigw = np.zeros((128, WN), np.float32)
    bigw[:, LW0:LW0 + 64] = f(inputs['lin0_w']).T
    bigw[:, LW1:LW1 + 64] = f(inputs['lin1_w']).T
    bigw[0:64, A1W:A1W + 32] = f(inputs['a0_1w']).T
    bigw[64:128, A1W + 32:A1W + 64] = f(inputs['a1_1w']).T
    bigw[0:32, A2W:A2W + 1] = f(inputs['a0_2w']).T
    bigw[32:64, A2W + 1:A2W + 2] = f(inputs['a1_2w']).T
    bigw[0, SEL:SEL + 64] = 1.0
    bigw[1, SEL + 64:SEL + 128] = 1.0
    bigw[:, BSTK] = b0
    bigw[:, BSTK + 1] = b1
    bigw[:, BONE] = 1.0

    bigr = np.zeros((128, NR), np.float32)
    bigr[:, CU0:CU0 + 128] = uw0
    bigr[:, CU1:CU1 + 128] = uw1
    bigf = np.zeros((128, NF32), np.float32)
    bigf[:, LINB] = np.concatenate([f(inputs['lin0_b']), f(inputs['lin1_b'])])
    bigf[0:64, A1B] = np.concatenate([f(inputs['a0_1b']), f(inputs['a1_1b'])])
    bigf[0:2, A2B] = np.concatenate([f(inputs['a0_2b']), f(inputs['a1_2b'])])

    cT = np.ascontiguousarray(f(inputs['c']).T)        # [64, 4096]

    in_maps = []
    for cid in range(N_CORES):
        g = cid // 4
        mw = bigw.copy()
        mw[:, ZWO:ZWO + OWN] = zwT[:, cid * OWN:(cid + 1) * OWN]
        mw[:, BOWN] = b0 if g == 0 else b1
        mr = bigr.copy()
        mr[:, ZWOF:ZWOF + OWN] = zwT[:, cid * OWN:(cid + 1) * OWN]
        mr[:, CUWO:CUWO + 128] = uw0 if g == 0 else uw1
        in_maps.append({
            'bigw': np.ascontiguousarray(mw.astype(BF)),
            'zwc': np.ascontiguousarray(zwc.astype(BF)),
            'bigr': np.ascontiguousarray(mr),
            'bigf': bigf,
            'cT': cT,
        })
    return in_maps


def kernel(**inputs):
    nc = _get_program()
    in_maps = make_in_maps(inputs)
    res = run_bass_kernel_spmd(nc, in_maps, list(range(N_CORES)))
    tot = 0.0
    for r in res.results:
        v = np.asarray(r['vout'], dtype=np.float64)
        se = v[:, 0:4] + v[:, 4:8]          # [128, 4]: sum over both groups
        dg = v[:, 8:12]
        tot += np.sum(dg - SHIFT - np.log(se))
    return np.array(-(tot / B), dtype=np.float32)


# revision 22
# speedup vs baseline: 1.0991x; 1.0233x over previous
"""Trainium2 Bass kernel for nn_CPCModel (CPC-style NCE loss).

Strategy (8 NeuronCores, full inputs on every core, no collectives):

The reference's leave-one-out softmax pooling collapses algebraically:
    pooled[i] = (T - e_i * zt_i) / (S - e_i),  e = exp(s), S = sum(e), T = sum(e_j zt_j)
so the [B,B] pooling matrix is never materialized.  The loss needs only
    nce = -mean_i( total[i,i] - logsumexp_j total[i,j] )
with  total[i, j in group g] = Azw_g[i]·pooled_g[j] + Czw[i]·c[j] + delta_g[i]
where Azw_g = zw @ Ww_g, Czw = zw @ Wk_w, delta_g = zw @ (Ww_g_b + Wk_b).

Each core redundantly computes the cheap pooling prep for all 4096 rows
(both groups stacked on the 128 partitions) and its own 512 rows of the
[4096,4096] total matrix + row-wise sum(exp(total - 44)) via 8
[128,2048] exp chunks on the scalar engine with accum_out.  The loop
rhs V2 = [cT ; pooled0|pooled1] is assembled by three DMAs (no compute)
and doubles as the diagonal's source through one partition_id-dynamic
column slice.  Raw per-row exp sums and raw diagonal values ship to the
host, which finishes with log() in float64.

Scheduling: weights arrive in a small first DMA and zw^T in two
chunk-major halves so the zt->h->s chain starts ~4us in; exp-of-s is
chunked so the fused ztw/T tensor_tensor_reduce starts before the last
tanh; U/bias matmuls and paced PE probes keep the tensor engine out of
its low p-state before the main loop.  fp32r matmuls keep K=128
throughout (K=64 fp32r dies at runtime).  One activation-table set
(exp_and_others: relu/tanh/exp/copy) serves the whole kernel; the final
log runs on the host.
"""

import numpy as np

import concourse.bacc as bacc
import concourse.bass as bass
import concourse.mybir as mybir
import concourse.tile as tile
from concourse.bass_utils import run_bass_kernel_spmd

N_CORES = 8
B = 4096
OWN = B // N_CORES            # 512 rows of `total` per core
G = 2048                      # group size
F32 = mybir.dt.float32
F32R = mybir.dt.float32r
BF16 = mybir.dt.bfloat16
AF = mybir.ActivationFunctionType
ALU = mybir.AluOpType
SHIFT = 44.0

# bf16 weights bundle column offsets
ZWO = 0           # zwoTb [128, 512]
LW0 = 512         # lin0_w.T [128, 64]
LW1 = 576         # lin1_w.T [128, 64]
A1W = 640         # blockdiag(a0_1w.T, a1_1w.T) [128, 64]
A2W = 704         # blockdiag(a0_2w.T, a1_2w.T) [64, 2]
SEL = 706         # sel2 [2, 128]
BSTK = 834        # [b0 | b1] [128, 2]
BOWN = 836        # b_{group(core)} [128, 1]
BONE = 837        # ones [128, 1]
WN = 838

# f32r bundle column offsets
ZWOF = 0          # zwoT [128, 512]
CU0 = 512         # hstack(Wk_w, Ww0_w) [128, 128]
CU1 = 640         # hstack(Wk_w, Ww1_w)
CUWO = 768        # per-core hstack(Wk_w, Ww_g) [128, 128] (diag)
NR = 896
# f32 bundle column offsets
LINB = 0          # [lin0_b ; lin1_b] [128, 1]
A1B = 1           # [a0_1b ; a1_1b] [64, 1]
A2B = 2           # [a0_2b ; a1_2b] [2, 1]
NF32 = 3


def _build_program(static_diag=False):
    nc = bacc.Bacc(
        "TRN2",
        target_bir_lowering=False,
        debug=False,
        num_devices=N_CORES,
    )

    bigw_d = nc.dram_tensor("bigw", [128, WN], BF16, kind="ExternalInput").ap()
    zwc_d = nc.dram_tensor("zwc", [128, B], BF16, kind="ExternalInput").ap()
    bigr_d = nc.dram_tensor("bigr", [128, NR], F32R, kind="ExternalInput").ap()
    bigf_d = nc.dram_tensor("bigf", [128, NF32], F32, kind="ExternalInput").ap()
    cT_d = nc.dram_tensor("cT", [64, B], F32R, kind="ExternalInput").ap()
    vout_d = nc.dram_tensor("vout", [128, 12], F32, kind="ExternalOutput").ap()

    from contextlib import ExitStack
    with tile.TileContext(nc) as tc, ExitStack() as ctx:
        pers = ctx.enter_context(tc.tile_pool(name="pers", bufs=1))
        scr = ctx.enter_context(tc.tile_pool(name="scr", bufs=2))

        # DMA order = need order: weights, zw halves, f32r/f32 bundles, cT
        bw = pers.tile([128, WN], BF16, tag="bw", name="bw")
        nc.sync.dma_start(bw[:], bigw_d[:])
        bf = pers.tile([128, NF32], F32, tag="bf", name="bf")
        nc.sync.dma_start(bf[:], bigf_d[:])
        zwc = pers.tile([128, B], BF16, tag="zwc", name="zwc")
        nc.sync.dma_start(zwc[:, 0:G], zwc_d[:, 0:G])
        nc.sync.dma_start(zwc[:, G:B], zwc_d[:, G:B])
        br = pers.tile([128, NR], F32R, tag="br", name="br")
        nc.sync.dma_start(br[:], bigr_d[:])
        # V2 = [cT ; pooled0|pooled1]: loop rhs AND diagonal source
        V2 = pers.tile([128, B], F32R, tag="V2", name="V2")
        nc.sync.dma_start(V2[0:64, :], cT_d[:])

        zwoT = br[:, ZWOF:ZWOF + 512]
        sel2 = bw[0:2, SEL:SEL + 128]
        linb2 = bf[:, LINB:LINB + 1]
        a1b2 = bf[0:64, A1B:A1B + 1]
        a2b2 = bf[0:2, A2B:A2B + 1]

        ztT2 = pers.tile([128, G], BF16, tag="ztT2")
        hT2 = pers.tile([64, G], BF16, tag="hT2")
        eT2 = pers.tile([2, G], BF16, tag="eT2")
        d2 = pers.tile([2, G], BF16, tag="d2")
        bT2 = pers.tile([2, G], BF16, tag="bT2")
        ztwT2 = pers.tile([128, G], F32, tag="ztwT2")
        pooled2 = pers.tile([128, G], F32R, tag="pooled2")
        Sacc = pers.tile([2, 4], F32, tag="Sacc")
        Tacc = pers.tile([128, 4], F32, tag="Tacc")
        T2 = pers.tile([128, 1], F32, tag="T2")
        S2 = pers.tile([2, 1], F32, tag="S2")
        biasS = pers.tile([128, 8], F32, tag="biasS")
        U0 = pers.tile([128, OWN], F32R, tag="U0")
        U1 = pers.tile([128, OWN], F32R, tag="U1")
        UOwnS = pers.tile([128, OWN], F32, tag="UOwnS")
        vout = pers.tile([128, 12], F32, tag="vout")

        actwarm = pers.tile([2, 1], BF16, tag="actwarm")

        with tc.tile_pool(name="psA", bufs=3, space="PSUM") as psA, \
             tc.tile_pool(name="psB", bufs=1, space="PSUM") as psB, \
             tc.tile_pool(name="psU", bufs=3, space="PSUM") as psU:
            # trigger the one-time activation table load before tanh needs it
            nc.scalar.activation(actwarm[:], bw[0:2, 0:1], AF.Tanh)
            # PE p-state warmups ahead of the zt chain
            for _ in range(2):
                pw0 = psU.tile([128, 512], F32, tag="pu")
                nc.tensor.matmul(pw0[:], bw[:, 0:128], bw[:, 0:512],
                                 start=True, stop=True)
            # ---- zt -> h -> s -> e chain (stacked groups, 4 x 512 cols) ----
            for ch in range(4):
                sl = slice(ch * 512, (ch + 1) * 512)
                pz = psA.tile([128, 512], F32, tag="ps")
                nc.tensor.matmul(pz[0:64, :], bw[:, LW0:LW0 + 64],
                                 zwc[:, ch * 1024:ch * 1024 + 512],
                                 start=True, stop=True)
                nc.tensor.matmul(pz[64:128, :], bw[:, LW1:LW1 + 64],
                                 zwc[:, ch * 1024 + 512:(ch + 1) * 1024],
                                 start=True, stop=True)
                # relu(x + bias) on DVE (GPSIMD can't read PSUM)
                nc.vector.tensor_scalar(ztT2[:, sl], pz[:], linb2, 0.0,
                                        op0=ALU.add, op1=ALU.max)
                ph = psA.tile([128, 512], F32, tag="ps")
                nc.tensor.matmul(ph[0:64, :], bw[:, A1W:A1W + 64], ztT2[:, sl],
                                 start=True, stop=True)
                nc.scalar.activation(hT2[:, sl], ph[0:64, :], AF.Tanh,
                                     bias=a1b2)
                ps_ = psA.tile([128, 512], F32, tag="ps")
                nc.tensor.matmul(ps_[0:2, :], bw[0:64, A2W:A2W + 2], hT2[:, sl],
                                 start=True, stop=True)
                nc.scalar.activation(eT2[:, sl], ps_[0:2, :], AF.Exp,
                                     bias=a2b2, accum_out=Sacc[:, ch:ch + 1])

            # ---- e-broadcast + fused ztw/T (DVE order: after all relus) ----
            for ch in range(4):
                sl = slice(ch * 512, (ch + 1) * 512)
                peb = psU.tile([128, 512], F32, tag="pu")
                nc.tensor.matmul(peb[:], sel2, eT2[:, sl], start=True, stop=True)
                nc.vector.tensor_tensor(ztwT2[:, sl], ztT2[:, sl], peb[:],
                                        op=ALU.mult)
                nc.vector.reduce_sum(Tacc[:, ch:ch + 1], ztwT2[:, sl],
                                     axis=mybir.AxisListType.X)

            # ---- beta = 1/(e - S) ----
            nc.vector.reduce_sum(S2[:], Sacc[:], axis=mybir.AxisListType.X)
            nc.vector.tensor_scalar(d2[:], eT2[:], S2[:], None,
                                    op0=ALU.subtract)
            for ch in range(4):
                sl = slice(ch * 512, (ch + 1) * 512)
                with nc.allow_low_precision(reason="beta in bf16 for PE bcast"):
                    nc.vector.reciprocal(bT2[:, sl], d2[:, sl])
            nc.vector.reduce_sum(T2[:], Tacc[:], axis=mybir.AxisListType.X)

            # ---- U tiles + delta biases (PE slack while beta computes) ----
            upus = []
            for uoff in (CU0, CU1):
                pu = psU.tile([128, 512], F32, tag="pu")
                nc.tensor.matmul(pu[:], br[:, uoff:uoff + 128],
                                 zwoT, start=True, stop=True)
                upus.append(pu)
            puo = psU.tile([128, 512], F32, tag="pu")
            nc.tensor.matmul(puo[:], br[:, CUWO:CUWO + 128],
                             zwoT, start=True, stop=True)
            nc.scalar.copy(UOwnS[:], puo[:])
            pbias = psB.tile([128, 8], F32, tag="pb")
            for ic in range(4):
                nc.tensor.matmul(pbias[:, ic * 2:ic * 2 + 2],
                                 bw[:, ZWO + ic * 128:ZWO + (ic + 1) * 128],
                                 bw[:, BSTK:BSTK + 2], start=True, stop=True)
            nc.vector.tensor_scalar(biasS[:], pbias[:], -SHIFT, None,
                                    op0=ALU.add)

            # ---- pooled = (ztw - T) * beta_bcast ----
            for ch in range(4):
                sl = slice(ch * 512, (ch + 1) * 512)
                pbb = psA.tile([128, 512], F32, tag="ps")
                nc.tensor.matmul(pbb[:], sel2, bT2[:, sl], start=True, stop=True)
                nc.vector.scalar_tensor_tensor(
                    out=pooled2[:, sl], in0=ztwT2[:, sl], scalar=T2[:],
                    in1=pbb[:], op0=ALU.subtract, op1=ALU.mult)
                # paced junk matmul: keeps the PE out of its low p-state
                # between the prep matmuls and the main loop
                pw = psA.tile([128, 512], F32, tag="ps")
                nc.tensor.matmul(pw[:], br[:, CU0:CU0 + 128],
                                 pooled2[:, sl], start=True, stop=True)

            # U copies last on DVE: needed only once the loop starts
            nc.vector.tensor_copy(U0[:], upus[0][:])
            nc.vector.tensor_copy(U1[:], upus[1][:])

        # V2 rows 64:128: cols 0:2048 = pooled0, cols 2048:4096 = pooled1
        nc.sync.dma_start(V2[64:128, 0:G], pooled2[0:64, :])
        nc.sync.dma_start(V2[64:128, G:B], pooled2[64:128, :])

        # ---- main loop: 8 chunks of [128 own rows, 2048 group cols] ----
        with tc.tile_pool(name="pbig", bufs=2, space="PSUM") as pbig:
            for g in range(2):
                Ug = U0 if g == 0 else U1
                for ic in range(4):
                    usl = slice(ic * 128, (ic + 1) * 128)
                    pm = pbig.tile([128, G], F32, tag="pb")
                    for q in range(4):
                        qs = slice(q * 512, (q + 1) * 512)
                        nc.tensor.matmul(pm[:, qs], Ug[:, usl],
                                         V2[:, g * G + q * 512:
                                            g * G + (q + 1) * 512],
                                         start=True, stop=True)
                    es = scr.tile([128, G], BF16, tag="es")
                    cc = g * 4 + ic
                    nc.scalar.activation(es[:], pm[:], AF.Exp,
                                         bias=biasS[:, 2 * ic + g:
                                                    2 * ic + g + 1],
                                         accum_out=vout[:, cc:cc + 1])

        # ---- diagonal: diag[i] = UOwn[:,i]·V2[:,own(i)] + delta ----
        with tc.tile_pool(name="ptail", bufs=1, space="PSUM") as pt:
            if static_diag:
                csl = slice(0, OWN)
            else:
                pid = nc.vector.partition_id()
                csl = bass.ts(pid, OWN)
            prod = pers.tile([128, OWN], BF16, tag="prod")
            nc.vector.tensor_tensor(prod[0:64, :], UOwnS[0:64, :],
                                    V2[0:64, csl].bitcast(F32), op=ALU.mult)
            nc.vector.tensor_tensor(prod[64:128, :], UOwnS[64:128, :],
                                    V2[64:128, csl].bitcast(F32),
                                    op=ALU.mult)
            pdg = pt.tile([128, 4], F32, tag="pt")
            for ic in range(4):
                nc.tensor.matmul(pdg[:, ic:ic + 1],
                                 prod[:, ic * 128:(ic + 1) * 128],
                                 bw[:, BONE:BONE + 1], start=True, stop=False)
                nc.tensor.matmul(pdg[:, ic:ic + 1],
                                 bw[:, ZWO + ic * 128:ZWO + (ic + 1) * 128],
                                 bw[:, BOWN:BOWN + 1], start=False, stop=True)
            nc.vector.tensor_copy(vout[:, 8:12], pdg[:])
            nc.sync.dma_start(vout_d[:], vout[:])

    nc.compile()
    return nc


_built = None


def _get_program():
    global _built
    if _built is None:
        _built = _build_program()
    return _built


def make_in_maps(inputs):
    import ml_dtypes
    BF = ml_dtypes.bfloat16
    f = lambda x: np.asarray(x, dtype=np.float32)

    zw = np.concatenate([f(inputs['zw_0']), f(inputs['zw_1'])], axis=0)
    zwT = np.ascontiguousarray(zw.T)                  # [128, 4096]
    # chunk-major layout: block ch = [group0 cols ch*512.. | group1 cols ...]
    zwc = np.empty_like(zwT)
    for ch in range(4):
        zwc[:, ch * 1024:ch * 1024 + 512] = zwT[:, ch * 512:(ch + 1) * 512]
        zwc[:, ch * 1024 + 512:(ch + 1) * 1024] = \
            zwT[:, G + ch * 512:G + (ch + 1) * 512]
    b0 = f(inputs['Ww0_b']) + f(inputs['Wk_b'])
    b1 = f(inputs['Ww1_b']) + f(inputs['Wk_b'])
    wk = f(inputs['Wk_w'])
    uw0 = np.hstack([wk, f(inputs['Ww0_w'])])          # [Czw ; Azw0]
    uw1 = np.hstack([wk, f(inputs['Ww1_w'])])          # [Czw ; Azw1]

    bigw = np.zeros((128, WN), np.float32)
    bigw[:, LW0:LW0 + 64] = f(inputs['lin0_w']).T
    bigw[:, LW1:LW1 + 64] = f(inputs['lin1_w']).T
    bigw[0:64, A1W:A1W + 32] = f(inputs['a0_1w']).T
    bigw[64:128, A1W + 32:A1W + 64] = f(inputs['a1_1w']).T
    bigw[0:32, A2W:A2W + 1] = f(inputs['a0_2w']).T
    bigw[32:64, A2W + 1:A2W + 2] = f(inputs['a1_2w']).T
    bigw[0, SEL:SEL + 64] = 1.0
    bigw[1, SEL + 64:SEL + 128] = 1.0
    bigw[:, BSTK] = b0
    bigw[:, BSTK + 1] = b1
    bigw[:, BONE] = 1.0

    bigr = np.zeros((128, NR), np.float32)
    bigr[:, CU0:CU0 + 128] = uw0
    bigr[:, CU1:CU1 + 128] = uw1
    bigf = np.zeros((128, NF32), np.float32)
    bigf[:, LINB] = np.concatenate([f(inputs['lin0_b']), f(inputs['lin1_b'])])
    bigf[0:64, A1B] = np.concatenate([f(inputs['a0_1b']), f(inputs['a1_1b'])])
    bigf[0:2, A2B] = np.concatenate([f(inputs['a0_2b']), f(inputs['a1_2b'])])

    cT = np.ascontiguousarray(f(inputs['c']).T)        # [64, 4096]

    in_maps = []
    for cid in range(N_CORES):
        g = cid // 4
        mw = bigw.copy()
        mw[:, ZWO:ZWO + OWN] = zwT[:, cid * OWN:(cid + 1) * OWN]
        mw[:, BOWN] = b0 if g == 0 else b1
        mr = bigr.copy()
        mr[:, ZWOF:ZWOF + OWN] = zwT[:, cid * OWN:(cid + 1) * OWN]
        mr[:, CUWO:CUWO + 128] = uw0 if g == 0 else uw1
        in_maps.append({
            'bigw': np.ascontiguousarray(mw.astype(BF)),
            'zwc': np.ascontiguousarray(zwc.astype(BF)),
            'bigr': np.ascontiguousarray(mr),
            'bigf': bigf,
            'cT': cT,
        })
    return in_maps


def kernel(**inputs):
    nc = _get_program()
    in_maps = make_in_maps(inputs)
    res = run_bass_kernel_spmd(nc, in_maps, list(range(N_CORES)))
    tot = 0.0
    for r in res.results:
        v = np.asarray(r['vout'], dtype=np.float64)
        se = v[:, 0:4] + v[:, 4:8]          # [128, 4]: sum over both groups
        dg = v[:, 8:12]
        tot += np.sum(dg - SHIFT - np.log(se))
    return np.array(-(tot / B), dtype=np.float32)


# revision 25
# speedup vs baseline: 1.1337x; 1.0314x over previous
"""Trainium2 Bass kernel for nn_CPCModel (CPC-style NCE loss).

Strategy (8 NeuronCores, full inputs on every core, no collectives):

The reference's leave-one-out softmax pooling collapses algebraically:
    pooled[i] = (T - e_i * zt_i) / (S - e_i),  e = exp(s), S = sum(e), T = sum(e_j zt_j)
so the [B,B] pooling matrix is never materialized.  The loss needs only
    nce = -mean_i( total[i,i] - logsumexp_j total[i,j] )
with  total[i, j in group g] = Azw_g[i]·pooled_g[j] + Czw[i]·c[j] + delta_g[i]
where Azw_g = zw @ Ww_g, Czw = zw @ Wk_w, delta_g = zw @ (Ww_g_b + Wk_b).

Each core redundantly computes the cheap pooling prep for all 4096 rows
(both groups stacked on the 128 partitions) and its own 512 rows of the
[4096,4096] total matrix + row-wise sum(exp(total - 44)) via 8
[128,2048] exp chunks on the scalar engine with accum_out.  The loop
rhs V2 = [cT ; pooled0|pooled1] is assembled by three DMAs (no compute)
and doubles as the diagonal's source through one partition_id-dynamic
column slice.  Raw per-row exp sums and raw diagonal values ship to the
host, which finishes with log() in float64.

Scheduling: weights arrive in a small first DMA and zw^T in two
chunk-major halves so the zt->h->s chain starts ~4us in; exp-of-s is
chunked so the fused ztw/T tensor_tensor_reduce starts before the last
tanh; U/bias matmuls and paced PE probes keep the tensor engine out of
its low p-state before the main loop.  fp32r matmuls keep K=128
throughout (K=64 fp32r dies at runtime).  One activation-table set
(exp_and_others: relu/tanh/exp/copy) serves the whole kernel; the final
log runs on the host.
"""

import numpy as np

import concourse.bacc as bacc
import concourse.bass as bass
import concourse.mybir as mybir
import concourse.tile as tile
from concourse.bass_utils import run_bass_kernel_spmd

N_CORES = 8
B = 4096
OWN = B // N_CORES            # 512 rows of `total` per core
G = 2048                      # group size
F32 = mybir.dt.float32
F32R = mybir.dt.float32r
BF16 = mybir.dt.bfloat16
AF = mybir.ActivationFunctionType
ALU = mybir.AluOpType
SHIFT = 44.0

# bf16 weights bundle column offsets
ZWO = 0           # zwoTb [128, 512]
LW0 = 512         # lin0_w.T [128, 64]
LW1 = 576         # lin1_w.T [128, 64]
A1W = 640         # blockdiag(a0_1w.T, a1_1w.T) [128, 64]
A2W = 704         # blockdiag(a0_2w.T, a1_2w.T) [64, 2]
SEL = 706         # sel2 [2, 128]
BSTK = 834        # [b0 | b1] [128, 2]
BOWN = 836        # b_{group(core)} [128, 1]
BONE = 837        # ones [128, 1]
WN = 838

# f32r bundle column offsets
ZWOF = 0          # zwoT [128, 512]
CU0 = 512         # hstack(Wk_w, Ww0_w) [128, 128]
CU1 = 640         # hstack(Wk_w, Ww1_w)
CUWO = 768        # per-core hstack(Wk_w, Ww_g) [128, 128] (diag)
NR = 896
# f32 bundle column offsets
LINB = 0          # [lin0_b ; lin1_b] [128, 1]
A1B = 1           # [a0_1b ; a1_1b] [64, 1]
A2B = 2           # [a0_2b ; a1_2b] [2, 1]
NF32 = 3


def _build_program(static_diag=False):
    nc = bacc.Bacc(
        "TRN2",
        target_bir_lowering=False,
        debug=False,
        num_devices=N_CORES,
    )

    bigw_d = nc.dram_tensor("bigw", [128, WN], BF16, kind="ExternalInput").ap()
    zwc_d = nc.dram_tensor("zwc", [128, B], BF16, kind="ExternalInput").ap()
    bigr_d = nc.dram_tensor("bigr", [128, NR], F32R, kind="ExternalInput").ap()
    bigf_d = nc.dram_tensor("bigf", [128, NF32], F32, kind="ExternalInput").ap()
    cT_d = nc.dram_tensor("cT", [64, B], F32R, kind="ExternalInput").ap()
    vout_d = nc.dram_tensor("vout", [128, 12], F32, kind="ExternalOutput").ap()

    from contextlib import ExitStack
    with tile.TileContext(nc) as tc, ExitStack() as ctx:
        pers = ctx.enter_context(tc.tile_pool(name="pers", bufs=1))
        scr = ctx.enter_context(tc.tile_pool(name="scr", bufs=2))

        # DMA order = need order: weights, zw halves, f32r/f32 bundles, cT
        bw = pers.tile([128, WN], BF16, tag="bw", name="bw")
        nc.sync.dma_start(bw[:], bigw_d[:])
        bf = pers.tile([128, NF32], F32, tag="bf", name="bf")
        nc.sync.dma_start(bf[:], bigf_d[:])
        zwc = pers.tile([128, B], BF16, tag="zwc", name="zwc")
        for p in range(4):
            nc.sync.dma_start(zwc[:, p * 1024:(p + 1) * 1024],
                              zwc_d[:, p * 1024:(p + 1) * 1024])
        br = pers.tile([128, NR], F32R, tag="br", name="br")
        nc.sync.dma_start(br[:], bigr_d[:])
        # V2 = [cT ; pooled0|pooled1]: loop rhs AND diagonal source
        V2 = pers.tile([128, B], F32R, tag="V2", name="V2")
        nc.sync.dma_start(V2[0:64, :], cT_d[:])

        zwoT = br[:, ZWOF:ZWOF + 512]
        sel2 = bw[0:2, SEL:SEL + 128]
        linb2 = bf[:, LINB:LINB + 1]
        a1b2 = bf[0:64, A1B:A1B + 1]
        a2b2 = bf[0:2, A2B:A2B + 1]

        ztT2 = pers.tile([128, G], BF16, tag="ztT2")
        hT2 = pers.tile([64, G], BF16, tag="hT2")
        eT2 = pers.tile([2, G], BF16, tag="eT2")
        d2 = pers.tile([2, G], BF16, tag="d2")
        bT2 = pers.tile([2, G], BF16, tag="bT2")
        ztwT2 = pers.tile([128, G], F32, tag="ztwT2")
        pooled2 = pers.tile([128, G], F32R, tag="pooled2")
        Sacc = pers.tile([2, 4], F32, tag="Sacc")
        Tacc = pers.tile([128, 4], F32, tag="Tacc")
        T2 = pers.tile([128, 1], F32, tag="T2")
        S2 = pers.tile([2, 1], F32, tag="S2")
        biasS = pers.tile([128, 8], F32, tag="biasS")
        U0 = pers.tile([128, OWN], F32R, tag="U0")
        U1 = pers.tile([128, OWN], F32R, tag="U1")
        UOwnS = pers.tile([128, OWN], F32, tag="UOwnS")
        vout = pers.tile([128, 12], F32, tag="vout")

        actwarm = pers.tile([2, 1], BF16, tag="actwarm")

        with tc.tile_pool(name="psA", bufs=6, space="PSUM") as psA, \
             tc.tile_pool(name="psB", bufs=1, space="PSUM") as psB:
            psU = psA
            # trigger the one-time activation table load before tanh needs it
            nc.scalar.activation(actwarm[:], bw[0:2, 0:1], AF.Tanh)
            # PE p-state warmups ahead of the zt chain
            for _ in range(4):
                pw0 = psA.tile([128, 512], F32, tag="ps")
                nc.tensor.matmul(pw0[:], bw[:, 0:128], bw[:, 0:512],
                                 start=True, stop=True)

            # ---- zt -> h -> s -> e chain, issued breadth-first so each
            # engine's in-order queue never blocks the next chunk ----
            SL = [slice(ch * 512, (ch + 1) * 512) for ch in range(4)]
            pzs = []
            for ch in range(4):
                pz = psA.tile([128, 512], F32, tag="ps")
                nc.tensor.matmul(pz[0:64, :], bw[:, LW0:LW0 + 64],
                                 zwc[:, ch * 1024:ch * 1024 + 512],
                                 start=True, stop=True)
                nc.tensor.matmul(pz[64:128, :], bw[:, LW1:LW1 + 64],
                                 zwc[:, ch * 1024 + 512:(ch + 1) * 1024],
                                 start=True, stop=True)
                pzs.append(pz)
            for ch in range(4):
                # relu(x + bias) on DVE (GPSIMD can't read PSUM)
                nc.vector.tensor_scalar(ztT2[:, SL[ch]], pzs[ch][:], linb2, 0.0,
                                        op0=ALU.add, op1=ALU.max)
            phs = []
            for ch in range(4):
                ph = psA.tile([128, 512], F32, tag="ps")
                nc.tensor.matmul(ph[0:64, :], bw[:, A1W:A1W + 64],
                                 ztT2[:, SL[ch]], start=True, stop=True)
                phs.append(ph)
            pss = []
            for ch in range(4):
                ps_ = psA.tile([128, 512], F32, tag="ps")
                pss.append(ps_)
            for ch in range(4):
                nc.scalar.activation(hT2[:, SL[ch]], phs[ch][0:64, :], AF.Tanh,
                                     bias=a1b2)
                nc.tensor.matmul(pss[ch][0:2, :], bw[0:64, A2W:A2W + 2],
                                 hT2[:, SL[ch]], start=True, stop=True)
                nc.scalar.activation(eT2[:, SL[ch]], pss[ch][0:2, :], AF.Exp,
                                     bias=a2b2, accum_out=Sacc[:, ch:ch + 1])

            # ---- e-broadcast, ztw on DVE, T-partials on Act (accum) ----
            pebs = []
            for ch in range(4):
                peb = psA.tile([128, 512], F32, tag="ps")
                nc.tensor.matmul(peb[:], sel2, eT2[:, SL[ch]],
                                 start=True, stop=True)
                pebs.append(peb)
            for ch in range(4):
                nc.vector.tensor_tensor(ztwT2[:, SL[ch]], ztT2[:, SL[ch]],
                                        pebs[ch][:], op=ALU.mult)
                tjunk = scr.tile([128, 512], BF16, tag="tj")
                nc.scalar.activation(tjunk[:], ztwT2[:, SL[ch]], AF.Copy,
                                     accum_out=Tacc[:, ch:ch + 1])

            # ---- beta = 1/(e - S) on DVE ----
            nc.vector.reduce_sum(S2[:], Sacc[:], axis=mybir.AxisListType.X)
            nc.vector.tensor_scalar(d2[:], eT2[:], S2[:], None,
                                    op0=ALU.subtract)
            for ch in range(4):
                with nc.allow_low_precision(reason="beta in bf16 for PE bcast"):
                    nc.vector.reciprocal(bT2[:, SL[ch]], d2[:, SL[ch]])
            nc.vector.reduce_sum(T2[:], Tacc[:], axis=mybir.AxisListType.X)

            # ---- U tiles + delta biases (PE slack while beta computes) ----
            upus = []
            for uoff in (CU0, CU1):
                pu = psA.tile([128, 512], F32, tag="ps")
                nc.tensor.matmul(pu[:], br[:, uoff:uoff + 128],
                                 zwoT, start=True, stop=True)
                upus.append(pu)
            puo = psA.tile([128, 512], F32, tag="ps")
            nc.tensor.matmul(puo[:], br[:, CUWO:CUWO + 128],
                             zwoT, start=True, stop=True)
            nc.scalar.copy(UOwnS[:], puo[:])
            pbias = psB.tile([128, 8], F32, tag="pb")
            for ic in range(4):
                nc.tensor.matmul(pbias[:, ic * 2:ic * 2 + 2],
                                 bw[:, ZWO + ic * 128:ZWO + (ic + 1) * 128],
                                 bw[:, BSTK:BSTK + 2], start=True, stop=True)
            nc.vector.tensor_scalar(biasS[:], pbias[:], -SHIFT, None,
                                    op0=ALU.add)

            # ---- pooled = (ztw - T) * beta_bcast ----
            pbbs = []
            for ch in range(4):
                pbb = psA.tile([128, 512], F32, tag="ps")
                nc.tensor.matmul(pbb[:], sel2, bT2[:, SL[ch]],
                                 start=True, stop=True)
                pbbs.append(pbb)
            for ch in range(4):
                nc.vector.scalar_tensor_tensor(
                    out=pooled2[:, SL[ch]], in0=ztwT2[:, SL[ch]], scalar=T2[:],
                    in1=pbbs[ch][:], op0=ALU.subtract, op1=ALU.mult)
            # paced junk matmuls: keep the PE out of its low p-state between
            # the prep matmuls and the main loop
            for ch in range(4):
                pw = psA.tile([128, 512], F32, tag="ps")
                nc.tensor.matmul(pw[:], br[:, CU0:CU0 + 128],
                                 pooled2[:, SL[ch]], start=True, stop=True)

            # U copies last on DVE: needed only once the loop starts
            nc.vector.tensor_copy(U1[:], upus[1][:])
            nc.vector.tensor_copy(U0[:], upus[0][:])

        # V2 rows 64:128: pooled1 first (the loop runs group 1 first, so
        # the pooled0 partition-shift hides behind four 2048-col exp chunks)
        nc.sync.dma_start(V2[64:128, G:B], pooled2[64:128, :])
        nc.sync.dma_start(V2[64:128, 0:G], pooled2[0:64, :])

        # ---- main loop: 8 chunks of [128 own rows, 2048 group cols] ----
        with tc.tile_pool(name="pbig", bufs=2, space="PSUM") as pbig:
            for g in (1, 0):
                Ug = U0 if g == 0 else U1
                for ic in range(4):
                    usl = slice(ic * 128, (ic + 1) * 128)
                    pm = pbig.tile([128, G], F32, tag="pb")
                    for q in range(4):
                        qs = slice(q * 512, (q + 1) * 512)
                        nc.tensor.matmul(pm[:, qs], Ug[:, usl],
                                         V2[:, g * G + q * 512:
                                            g * G + (q + 1) * 512],
                                         start=True, stop=True)
                    es = scr.tile([128, G], BF16, tag="es")
                    cc = g * 4 + ic
                    nc.scalar.activation(es[:], pm[:], AF.Exp,
                                         bias=biasS[:, 2 * ic + g:
                                                    2 * ic + g + 1],
                                         accum_out=vout[:, cc:cc + 1])

        # ---- diagonal: diag[i] = UOwn[:,i]·V2[:,own(i)] + delta ----
        with tc.tile_pool(name="ptail", bufs=1, space="PSUM") as pt:
            if static_diag:
                csl = slice(0, OWN)
            else:
                pid = nc.vector.partition_id()
                csl = bass.ts(pid, OWN)
            prod = pers.tile([128, OWN], BF16, tag="prod")
            nc.vector.tensor_tensor(prod[0:64, :], UOwnS[0:64, :],
                                    V2[0:64, csl].bitcast(F32), op=ALU.mult)
            nc.vector.tensor_tensor(prod[64:128, :], UOwnS[64:128, :],
                                    V2[64:128, csl].bitcast(F32),
                                    op=ALU.mult)
            pdg = pt.tile([128, 4], F32, tag="pt")
            for ic in range(4):
                nc.tensor.matmul(pdg[:, ic:ic + 1],
                                 prod[:, ic * 128:(ic + 1) * 128],
                                 bw[:, BONE:BONE + 1], start=True, stop=False)
                nc.tensor.matmul(pdg[:, ic:ic + 1],
                                 bw[:, ZWO + ic * 128:ZWO + (ic + 1) * 128],
                                 bw[:, BOWN:BOWN + 1], start=False, stop=True)
            nc.vector.tensor_copy(vout[:, 8:12], pdg[:])
            nc.sync.dma_start(vout_d[:], vout[:])

    nc.compile()
    return nc


_built = None


def _get_program():
    global _built
    if _built is None:
        _built = _build_program()
    return _built


def make_in_maps(inputs):
    import ml_dtypes
    BF = ml_dtypes.bfloat16
    f = lambda x: np.asarray(x, dtype=np.float32)

    zw = np.concatenate([f(inputs['zw_0']), f(inputs['zw_1'])], axis=0)
    zwT = np.ascontiguousarray(zw.T)                  # [128, 4096]
    # chunk-major layout: block ch = [group0 cols ch*512.. | group1 cols ...]
    zwc = np.empty_like(zwT)
    for ch in range(4):
        zwc[:, ch * 1024:ch * 1024 + 512] = zwT[:, ch * 512:(ch + 1) * 512]
        zwc[:, ch * 1024 + 512:(ch + 1) * 1024] = \
            zwT[:, G + ch * 512:G + (ch + 1) * 512]
    b0 = f(inputs['Ww0_b']) + f(inputs['Wk_b'])
    b1 = f(inputs['Ww1_b']) + f(inputs['Wk_b'])
    wk = f(inputs['Wk_w'])
    uw0 = np.hstack([wk, f(inputs['Ww0_w'])])          # [Czw ; Azw0]
    uw1 = np.hstack([wk, f(inputs['Ww1_w'])])          # [Czw ; Azw1]

    bigw = np.zeros((128, WN), np.float32)
    bigw[:, LW0:LW0 + 64] = f(inputs['lin0_w']).T
    bigw[:, LW1:LW1 + 64] = f(inputs['lin1_w']).T
    bigw[0:64, A1W:A1W + 32] = f(inputs['a0_1w']).T
    bigw[64:128, A1W + 32:A1W + 64] = f(inputs['a1_1w']).T
    bigw[0:32, A2W:A2W + 1] = f(inputs['a0_2w']).T
    bigw[32:64, A2W + 1:A2W + 2] = f(inputs['a1_2w']).T
    bigw[0, SEL:SEL + 64] = 1.0
    bigw[1, SEL + 64:SEL + 128] = 1.0
    bigw[:, BSTK] = b0
    bigw[:, BSTK + 1] = b1
    bigw[:, BONE] = 1.0

    bigr = np.zeros((128, NR), np.float32)
    bigr[:, CU0:CU0 + 128] = uw0
    bigr[:, CU1:CU1 + 128] = uw1
    bigf = np.zeros((128, NF32), np.float32)
    bigf[:, LINB] = np.concatenate([f(inputs['lin0_b']), f(inputs['lin1_b'])])
    bigf[0:64, A1B] = np.concatenate([f(inputs['a0_1b']), f(inputs['a1_1b'])])
    bigf[0:2, A2B] = np.concatenate([f(inputs['a0_2b']), f(inputs['a1_2b'])])

    cT = np.ascontiguousarray(f(inputs['c']).T)        # [64, 4096]

    in_maps = []
    for cid in range(N_CORES):
        g = cid // 4
        mw = bigw.copy()
        mw[:, ZWO:ZWO + OWN] = zwT[:, cid * OWN:(cid + 1) * OWN]
        mw[:, BOWN] = b0 if g == 0 else b1
        mr = bigr.copy()
        mr[:, ZWOF:ZWOF + OWN] = zwT[:, cid * OWN:(cid + 1) * OWN]
        mr[:, CUWO:CUWO + 128] = uw0 if g == 0 else uw1
        in_maps.append({
            'bigw': np.ascontiguousarray(mw.astype(BF)),
            'zwc': np.ascontiguousarray(zwc.astype(BF)),
            'bigr': np.ascontiguousarray(mr),
            'bigf': bigf,
            'cT': cT,
        })
    return in_maps


def kernel(**inputs):
    nc = _get_program()
    in_maps = make_in_maps(inputs)
    res = run_bass_kernel_spmd(nc, in_maps, list(range(N_CORES)))
    tot = 0.0
    for r in res.results:
        v = np.asarray(r['vout'], dtype=np.float64)
        se = v[:, 0:4] + v[:, 4:8]          # [128, 4]: sum over both groups
        dg = v[:, 8:12]
        tot += np.sum(dg - SHIFT - np.log(se))
    return np.array(-(tot / B), dtype=np.float32)


# revision 26
# speedup vs baseline: 1.1738x; 1.0354x over previous
"""Trainium2 Bass kernel for nn_CPCModel (CPC-style NCE loss).

Strategy (8 NeuronCores, full inputs on every core, no collectives):

The reference's leave-one-out softmax pooling collapses algebraically:
    pooled[i] = (T - e_i * zt_i) / (S - e_i),  e = exp(s), S = sum(e), T = sum(e_j zt_j)
so the [B,B] pooling matrix is never materialized.  The loss needs only
    nce = -mean_i( total[i,i] - logsumexp_j total[i,j] )
with  total[i, j in group g] = Azw_g[i]·pooled_g[j] + Czw[i]·c[j] + delta_g[i]
where Azw_g = zw @ Ww_g, Czw = zw @ Wk_w, delta_g = zw @ (Ww_g_b + Wk_b).

Each core redundantly computes the cheap pooling prep for all 4096 rows
(both groups stacked on the 128 partitions) and its own 512 rows of the
[4096,4096] total matrix + row-wise sum(exp(total - 44)) via 8
[128,2048] exp chunks on the scalar engine with accum_out.  The loop
rhs V2 = [cT ; pooled0|pooled1] is assembled by three DMAs (no compute)
and doubles as the diagonal's source through one partition_id-dynamic
column slice.  Raw per-row exp sums and raw diagonal values ship to the
host, which finishes with log() in float64.

Scheduling: weights arrive in a small first DMA and zw^T in two
chunk-major halves so the zt->h->s chain starts ~4us in; exp-of-s is
chunked so the fused ztw/T tensor_tensor_reduce starts before the last
tanh; U/bias matmuls and paced PE probes keep the tensor engine out of
its low p-state before the main loop.  fp32r matmuls keep K=128
throughout (K=64 fp32r dies at runtime).  One activation-table set
(exp_and_others: relu/tanh/exp/copy) serves the whole kernel; the final
log runs on the host.
"""

import numpy as np

import concourse.bacc as bacc
import concourse.bass as bass
import concourse.mybir as mybir
import concourse.tile as tile
from concourse.bass_utils import run_bass_kernel_spmd

N_CORES = 8
B = 4096
OWN = B // N_CORES            # 512 rows of `total` per core
G = 2048                      # group size
F32 = mybir.dt.float32
F32R = mybir.dt.float32r
BF16 = mybir.dt.bfloat16
AF = mybir.ActivationFunctionType
ALU = mybir.AluOpType
SHIFT = 44.0

# bf16 weights bundle column offsets
ZWO = 0           # zwoTb [128, 512]
LW0 = 512         # lin0_w.T [128, 64]
LW1 = 576         # lin1_w.T [128, 64]
A1W = 640         # blockdiag(a0_1w.T, a1_1w.T) [128, 64]
A2W = 704         # blockdiag(a0_2w.T, a1_2w.T) [64, 2]
SEL = 706         # sel2 [2, 128]
BSTK = 834        # [b0 | b1] [128, 2]
BOWN = 836        # b_{group(core)} [128, 1]
BONE = 837        # ones [128, 1]
WN = 838

# f32r bundle column offsets
ZWOF = 0          # zwoT [128, 512]
CU0 = 512         # hstack(Wk_w, Ww0_w) [128, 128]
CU1 = 640         # hstack(Wk_w, Ww1_w)
CUWO = 768        # per-core hstack(Wk_w, Ww_g) [128, 128] (diag)
NR = 896
# f32 bundle column offsets
LINB = 0          # [lin0_b ; lin1_b] [128, 1]
A1B = 1           # [a0_1b ; a1_1b] [64, 1]
A2B = 2           # [a0_2b ; a1_2b] [2, 1]
NF32 = 3


def _build_program(static_diag=False):
    nc = bacc.Bacc(
        "TRN2",
        target_bir_lowering=False,
        debug=False,
        num_devices=N_CORES,
    )

    bigw_d = nc.dram_tensor("bigw", [128, WN], BF16, kind="ExternalInput").ap()
    zwc_d = nc.dram_tensor("zwc", [128, B], BF16, kind="ExternalInput").ap()
    bigr_d = nc.dram_tensor("bigr", [128, NR], F32R, kind="ExternalInput").ap()
    bigf_d = nc.dram_tensor("bigf", [128, NF32], F32, kind="ExternalInput").ap()
    cT_d = nc.dram_tensor("cT", [64, B], F32R, kind="ExternalInput").ap()
    vout_d = nc.dram_tensor("vout", [128, 12], F32, kind="ExternalOutput").ap()

    from contextlib import ExitStack
    with tile.TileContext(nc) as tc, ExitStack() as ctx:
        pers = ctx.enter_context(tc.tile_pool(name="pers", bufs=1))
        scr = ctx.enter_context(tc.tile_pool(name="scr", bufs=2))

        # DMA order = need order: weights, zw halves, f32r/f32 bundles, cT
        bw = pers.tile([128, WN], BF16, tag="bw", name="bw")
        nc.sync.dma_start(bw[:], bigw_d[:])
        bf = pers.tile([128, NF32], F32, tag="bf", name="bf")
        nc.sync.dma_start(bf[:], bigf_d[:])
        zwc = pers.tile([128, B], BF16, tag="zwc", name="zwc")
        for p in range(4):
            nc.sync.dma_start(zwc[:, p * 1024:(p + 1) * 1024],
                              zwc_d[:, p * 1024:(p + 1) * 1024])
        br = pers.tile([128, NR], F32R, tag="br", name="br")
        nc.sync.dma_start(br[:], bigr_d[:])
        # V2 = [cT ; pooled0|pooled1]: loop rhs AND diagonal source
        V2 = pers.tile([128, B], F32R, tag="V2", name="V2")
        nc.sync.dma_start(V2[0:64, :], cT_d[:])

        zwoT = br[:, ZWOF:ZWOF + 512]
        sel2 = bw[0:2, SEL:SEL + 128]
        linb2 = bf[:, LINB:LINB + 1]
        a1b2 = bf[0:64, A1B:A1B + 1]
        a2b2 = bf[0:2, A2B:A2B + 1]

        ztT2 = pers.tile([128, G], BF16, tag="ztT2")
        hT2 = pers.tile([64, G], BF16, tag="hT2")
        eT2 = pers.tile([2, G], BF16, tag="eT2")
        d2 = pers.tile([2, G], BF16, tag="d2")
        bT2 = pers.tile([2, G], BF16, tag="bT2")
        ztwT2 = pers.tile([128, G], F32, tag="ztwT2")
        pooled2 = pers.tile([128, G], F32R, tag="pooled2")
        Sacc = pers.tile([2, 4], F32, tag="Sacc")
        Tacc = pers.tile([128, 4], F32, tag="Tacc")
        T2 = pers.tile([128, 1], F32, tag="T2")
        S2 = pers.tile([2, 1], F32, tag="S2")
        biasS = pers.tile([128, 8], F32, tag="biasS")
        U0 = pers.tile([128, OWN], F32R, tag="U0")
        U1 = pers.tile([128, OWN], F32R, tag="U1")
        UOwnS = pers.tile([128, OWN], F32, tag="UOwnS")
        vout = pers.tile([128, 12], F32, tag="vout")

        actwarm = pers.tile([2, 1], BF16, tag="actwarm")

        with tc.tile_pool(name="psA", bufs=6, space="PSUM") as psA, \
             tc.tile_pool(name="psB", bufs=1, space="PSUM") as psB:
            psU = psA
            # trigger the one-time activation table load before tanh needs it
            nc.scalar.activation(actwarm[:], bw[0:2, 0:1], AF.Tanh)
            # PE p-state warmups ahead of the zt chain
            for _ in range(6):
                pw0 = psA.tile([128, 512], F32, tag="ps")
                nc.tensor.matmul(pw0[:], bw[:, 0:128], bw[:, 0:512],
                                 start=True, stop=True)

            # ---- zt -> h -> s -> e chain, issued breadth-first so each
            # engine's in-order queue never blocks the next chunk ----
            SL = [slice(ch * 512, (ch + 1) * 512) for ch in range(4)]
            pzs = []
            for ch in range(4):
                pz = psA.tile([128, 512], F32, tag="ps")
                nc.tensor.matmul(pz[0:64, :], bw[:, LW0:LW0 + 64],
                                 zwc[:, ch * 1024:ch * 1024 + 512],
                                 start=True, stop=True)
                nc.tensor.matmul(pz[64:128, :], bw[:, LW1:LW1 + 64],
                                 zwc[:, ch * 1024 + 512:(ch + 1) * 1024],
                                 start=True, stop=True)
                pzs.append(pz)
            for ch in range(4):
                # relu(x + bias) on DVE (GPSIMD can't read PSUM)
                nc.vector.tensor_scalar(ztT2[:, SL[ch]], pzs[ch][:], linb2, 0.0,
                                        op0=ALU.add, op1=ALU.max)
            phs = []
            for ch in range(4):
                ph = psA.tile([128, 512], F32, tag="ps")
                nc.tensor.matmul(ph[0:64, :], bw[:, A1W:A1W + 64],
                                 ztT2[:, SL[ch]], start=True, stop=True)
                phs.append(ph)
            pss = []
            for ch in range(4):
                ps_ = psA.tile([128, 512], F32, tag="ps")
                pss.append(ps_)
            for ch in range(4):
                nc.scalar.activation(hT2[:, SL[ch]], phs[ch][0:64, :], AF.Tanh,
                                     bias=a1b2)
                nc.tensor.matmul(pss[ch][0:2, :], bw[0:64, A2W:A2W + 2],
                                 hT2[:, SL[ch]], start=True, stop=True)
                nc.scalar.activation(eT2[:, SL[ch]], pss[ch][0:2, :], AF.Exp,
                                     bias=a2b2, accum_out=Sacc[:, ch:ch + 1])

            # ---- e-broadcast, ztw on DVE, T-partials on Act (accum) ----
            pebs = []
            for ch in range(4):
                peb = psA.tile([128, 512], F32, tag="ps")
                nc.tensor.matmul(peb[:], sel2, eT2[:, SL[ch]],
                                 start=True, stop=True)
                pebs.append(peb)
            for ch in range(4):
                nc.vector.tensor_tensor(ztwT2[:, SL[ch]], ztT2[:, SL[ch]],
                                        pebs[ch][:], op=ALU.mult)
                tjunk = scr.tile([128, 512], BF16, tag="tj")
                nc.scalar.activation(tjunk[:], ztwT2[:, SL[ch]], AF.Copy,
                                     accum_out=Tacc[:, ch:ch + 1])

            # ---- beta = 1/(e - S) on DVE ----
            nc.vector.reduce_sum(S2[:], Sacc[:], axis=mybir.AxisListType.X)
            nc.vector.tensor_scalar(d2[:], eT2[:], S2[:], None,
                                    op0=ALU.subtract)
            for ch in range(4):
                with nc.allow_low_precision(reason="beta in bf16 for PE bcast"):
                    nc.vector.reciprocal(bT2[:, SL[ch]], d2[:, SL[ch]])
            nc.vector.reduce_sum(T2[:], Tacc[:], axis=mybir.AxisListType.X)

            # ---- U tiles + delta biases (PE slack while beta computes) ----
            upus = []
            for uoff in (CU0, CU1):
                pu = psA.tile([128, 512], F32, tag="ps")
                nc.tensor.matmul(pu[:], br[:, uoff:uoff + 128],
                                 zwoT, start=True, stop=True)
                upus.append(pu)
            puo = psA.tile([128, 512], F32, tag="ps")
            nc.tensor.matmul(puo[:], br[:, CUWO:CUWO + 128],
                             zwoT, start=True, stop=True)
            nc.scalar.copy(UOwnS[:], puo[:])
            pbias = psB.tile([128, 8], F32, tag="pb")
            for ic in range(4):
                nc.tensor.matmul(pbias[:, ic * 2:ic * 2 + 2],
                                 bw[:, ZWO + ic * 128:ZWO + (ic + 1) * 128],
                                 bw[:, BSTK:BSTK + 2], start=True, stop=True)
            nc.vector.tensor_scalar(biasS[:], pbias[:], -SHIFT, None,
                                    op0=ALU.add)

            # ---- pooled = (ztw - T) * beta_bcast ----
            pbbs = []
            for ch in range(4):
                pbb = psA.tile([128, 512], F32, tag="ps")
                nc.tensor.matmul(pbb[:], sel2, bT2[:, SL[ch]],
                                 start=True, stop=True)
                pbbs.append(pbb)
            for ch in range(4):
                nc.vector.scalar_tensor_tensor(
                    out=pooled2[:, SL[ch]], in0=ztwT2[:, SL[ch]], scalar=T2[:],
                    in1=pbbs[ch][:], op0=ALU.subtract, op1=ALU.mult)
                # V2 assembly piece-DMAs: group1 (loop runs it first), group0
                nc.sync.dma_start(V2[64:128, G + ch * 512:G + (ch + 1) * 512],
                                  pooled2[64:128, SL[ch]])
                nc.sync.dma_start(V2[64:128, SL[ch]], pooled2[0:64, SL[ch]])
            # paced junk matmuls: keep the PE out of its low p-state between
            # the prep matmuls and the main loop
            for ch in range(4):
                pw = psA.tile([128, 512], F32, tag="ps")
                nc.tensor.matmul(pw[:], br[:, CU0:CU0 + 128],
                                 pooled2[:, SL[ch]], start=True, stop=True)

            # U copies last on DVE: needed only once the loop starts
            nc.vector.tensor_copy(U1[:], upus[1][:])
            nc.vector.tensor_copy(U0[:], upus[0][:])

        # ---- main loop: 8 chunks of [128 own rows, 2048 group cols] ----
        with tc.tile_pool(name="pbig", bufs=2, space="PSUM") as pbig:
            for g in (1, 0):
                Ug = U0 if g == 0 else U1
                for ic in range(4):
                    usl = slice(ic * 128, (ic + 1) * 128)
                    pm = pbig.tile([128, G], F32, tag="pb")
                    for q in range(4):
                        qs = slice(q * 512, (q + 1) * 512)
                        nc.tensor.matmul(pm[:, qs], Ug[:, usl],
                                         V2[:, g * G + q * 512:
                                            g * G + (q + 1) * 512],
                                         start=True, stop=True)
                    es = scr.tile([128, G], BF16, tag="es")
                    cc = g * 4 + ic
                    nc.scalar.activation(es[:], pm[:], AF.Exp,
                                         bias=biasS[:, 2 * ic + g:
                                                    2 * ic + g + 1],
                                         accum_out=vout[:, cc:cc + 1])

        # ---- diagonal: diag[i] = UOwn[:,i]·V2[:,own(i)] + delta ----
        with tc.tile_pool(name="ptail", bufs=1, space="PSUM") as pt:
            if static_diag:
                csl = slice(0, OWN)
            else:
                pid = nc.vector.partition_id()
                csl = bass.ts(pid, OWN)
            prod = pers.tile([128, OWN], BF16, tag="prod")
            nc.vector.tensor_tensor(prod[0:64, :], UOwnS[0:64, :],
                                    V2[0:64, csl].bitcast(F32), op=ALU.mult)
            nc.vector.tensor_tensor(prod[64:128, :], UOwnS[64:128, :],
                                    V2[64:128, csl].bitcast(F32),
                                    op=ALU.mult)
            pdg = pt.tile([128, 4], F32, tag="pt")
            for ic in range(4):
                nc.tensor.matmul(pdg[:, ic:ic + 1],
                                 prod[:, ic * 128:(ic + 1) * 128],
                                 bw[:, BONE:BONE + 1], start=True, stop=False)
                nc.tensor.matmul(pdg[:, ic:ic + 1],
                                 bw[:, ZWO + ic * 128:ZWO + (ic + 1) * 128],
                                 bw[:, BOWN:BOWN + 1], start=False, stop=True)
            nc.vector.tensor_copy(vout[:, 8:12], pdg[:])
            nc.sync.dma_start(vout_d[:], vout[:])

    nc.compile()
    return nc


_built = None


def _get_program():
    global _built
    if _built is None:
        _built = _build_program()
    return _built


def make_in_maps(inputs):
    import ml_dtypes
    BF = ml_dtypes.bfloat16
    f = lambda x: np.asarray(x, dtype=np.float32)

    zw = np.concatenate([f(inputs['zw_0']), f(inputs['zw_1'])], axis=0)
    zwT = np.ascontiguousarray(zw.T)                  # [128, 4096]
    # chunk-major layout: block ch = [group0 cols ch*512.. | group1 cols ...]
    zwc = np.empty_like(zwT)
    for ch in range(4):
        zwc[:, ch * 1024:ch * 1024 + 512] = zwT[:, ch * 512:(ch + 1) * 512]
        zwc[:, ch * 1024 + 512:(ch + 1) * 1024] = \
            zwT[:, G + ch * 512:G + (ch + 1) * 512]
    b0 = f(inputs['Ww0_b']) + f(inputs['Wk_b'])
    b1 = f(inputs['Ww1_b']) + f(inputs['Wk_b'])
    wk = f(inputs['Wk_w'])
    uw0 = np.hstack([wk, f(inputs['Ww0_w'])])          # [Czw ; Azw0]
    uw1 = np.hstack([wk, f(inputs['Ww1_w'])])          # [Czw ; Azw1]

    bigw = np.zeros((128, WN), np.float32)
    bigw[:, LW0:LW0 + 64] = f(inputs['lin0_w']).T
    bigw[:, LW1:LW1 + 64] = f(inputs['lin1_w']).T
    bigw[0:64, A1W:A1W + 32] = f(inputs['a0_1w']).T
    bigw[64:128, A1W + 32:A1W + 64] = f(inputs['a1_1w']).T
    bigw[0:32, A2W:A2W + 1] = f(inputs['a0_2w']).T
    bigw[32:64, A2W + 1:A2W + 2] = f(inputs['a1_2w']).T
    bigw[0, SEL:SEL + 64] = 1.0
    bigw[1, SEL + 64:SEL + 128] = 1.0
    bigw[:, BSTK] = b0
    bigw[:, BSTK + 1] = b1
    bigw[:, BONE] = 1.0

    bigr = np.zeros((128, NR), np.float32)
    bigr[:, CU0:CU0 + 128] = uw0
    bigr[:, CU1:CU1 + 128] = uw1
    bigf = np.zeros((128, NF32), np.float32)
    bigf[:, LINB] = np.concatenate([f(inputs['lin0_b']), f(inputs['lin1_b'])])
    bigf[0:64, A1B] = np.concatenate([f(inputs['a0_1b']), f(inputs['a1_1b'])])
    bigf[0:2, A2B] = np.concatenate([f(inputs['a0_2b']), f(inputs['a1_2b'])])

    cT = np.ascontiguousarray(f(inputs['c']).T)        # [64, 4096]

    in_maps = []
    for cid in range(N_CORES):
        g = cid // 4
        mw = bigw.copy()
        mw[:, ZWO:ZWO + OWN] = zwT[:, cid * OWN:(cid + 1) * OWN]
        mw[:, BOWN] = b0 if g == 0 else b1
        mr = bigr.copy()
        mr[:, ZWOF:ZWOF + OWN] = zwT[:, cid * OWN:(cid + 1) * OWN]
        mr[:, CUWO:CUWO + 128] = uw0 if g == 0 else uw1
        in_maps.append({
            'bigw': np.ascontiguousarray(mw.astype(BF)),
            'zwc': np.ascontiguousarray(zwc.astype(BF)),
            'bigr': np.ascontiguousarray(mr),
            'bigf': bigf,
            'cT': cT,
        })
    return in_maps


def kernel(**inputs):
    nc = _get_program()
    in_maps = make_in_maps(inputs)
    res = run_bass_kernel_spmd(nc, in_maps, list(range(N_CORES)))
    tot = 0.0
    for r in res.results:
        v = np.asarray(r['vout'], dtype=np.float64)
        se = v[:, 0:4] + v[:, 4:8]          # [128, 4]: sum over both groups
        dg = v[:, 8:12]
        tot += np.sum(dg - SHIFT - np.log(se))
    return np.array(-(tot / B), dtype=np.float32)


# revision 27
# speedup vs baseline: 1.2206x; 1.0399x over previous
"""Trainium2 Bass kernel for nn_CPCModel (CPC-style NCE loss).

Strategy (8 NeuronCores, full inputs on every core, no collectives):

The reference's leave-one-out softmax pooling collapses algebraically:
    pooled[i] = (T - e_i * zt_i) / (S - e_i),  e = exp(s), S = sum(e), T = sum(e_j zt_j)
so the [B,B] pooling matrix is never materialized.  The loss needs only
    nce = -mean_i( total[i,i] - logsumexp_j total[i,j] )
with  total[i, j in group g] = Azw_g[i]·pooled_g[j] + Czw[i]·c[j] + delta_g[i]
where Azw_g = zw @ Ww_g, Czw = zw @ Wk_w, delta_g = zw @ (Ww_g_b + Wk_b).

Each core redundantly computes the cheap pooling prep for all 4096 rows
(both groups stacked on the 128 partitions) and its own 512 rows of the
[4096,4096] total matrix + row-wise sum(exp(total - 44)) via 8
[128,2048] exp chunks on the scalar engine with accum_out.  The loop
rhs V2 = [cT ; pooled0|pooled1] is assembled by three DMAs (no compute)
and doubles as the diagonal's source through one partition_id-dynamic
column slice.  Raw per-row exp sums and raw diagonal values ship to the
host, which finishes with log() in float64.

Scheduling: weights arrive in a small first DMA and zw^T in two
chunk-major halves so the zt->h->s chain starts ~4us in; exp-of-s is
chunked so the fused ztw/T tensor_tensor_reduce starts before the last
tanh; U/bias matmuls and paced PE probes keep the tensor engine out of
its low p-state before the main loop.  fp32r matmuls keep K=128
throughout (K=64 fp32r dies at runtime).  One activation-table set
(exp_and_others: relu/tanh/exp/copy) serves the whole kernel; the final
log runs on the host.
"""

import numpy as np

import concourse.bacc as bacc
import concourse.bass as bass
import concourse.mybir as mybir
import concourse.tile as tile
from concourse.bass_utils import run_bass_kernel_spmd

N_CORES = 8
B = 4096
OWN = B // N_CORES            # 512 rows of `total` per core
G = 2048                      # group size
F32 = mybir.dt.float32
F32R = mybir.dt.float32r
BF16 = mybir.dt.bfloat16
AF = mybir.ActivationFunctionType
ALU = mybir.AluOpType
SHIFT = 44.0

# bf16 weights bundle column offsets
ZWO = 0           # zwoTb [128, 512]
LW0 = 512         # lin0_w.T [128, 64]
LW1 = 576         # lin1_w.T [128, 64]
A1W = 640         # blockdiag(a0_1w.T, a1_1w.T) [128, 64]
A2W = 704         # blockdiag(a0_2w.T, a1_2w.T) [64, 2]
SEL = 706         # sel2 [2, 128]
BSTK = 834        # [b0 | b1] [128, 2]
BOWN = 836        # b_{group(core)} [128, 1]
BONE = 837        # ones [128, 1]
WN = 838

# f32r bundle column offsets
ZWOF = 0          # zwoT [128, 512]
CU0 = 512         # hstack(Wk_w, Ww0_w) [128, 128]
CU1 = 640         # hstack(Wk_w, Ww1_w)
CUWO = 768        # per-core hstack(Wk_w, Ww_g) [128, 128] (diag)
NR = 896
# f32 bundle column offsets
LINB = 0          # [lin0_b ; lin1_b] [128, 1]
A1B = 1           # [a0_1b ; a1_1b] [64, 1]
A2B = 2           # [a0_2b ; a1_2b] [2, 1]
NF32 = 3


def _build_program(static_diag=False):
    nc = bacc.Bacc(
        "TRN2",
        target_bir_lowering=False,
        debug=False,
        num_devices=N_CORES,
    )

    bigw_d = nc.dram_tensor("bigw", [128, WN], BF16, kind="ExternalInput").ap()
    zwc_d = nc.dram_tensor("zwc", [128, B], BF16, kind="ExternalInput").ap()
    bigr_d = nc.dram_tensor("bigr", [128, NR], F32R, kind="ExternalInput").ap()
    bigf_d = nc.dram_tensor("bigf", [128, NF32], F32, kind="ExternalInput").ap()
    cT_d = nc.dram_tensor("cT", [64, B], F32R, kind="ExternalInput").ap()
    vout_d = nc.dram_tensor("vout", [128, 12], F32, kind="ExternalOutput").ap()

    from contextlib import ExitStack
    with tile.TileContext(nc) as tc, ExitStack() as ctx:
        pers = ctx.enter_context(tc.tile_pool(name="pers", bufs=1))
        scr = ctx.enter_context(tc.tile_pool(name="scr", bufs=2))

        # DMA order = need order: weights, zw halves, f32r/f32 bundles, cT
        bw = pers.tile([128, WN], BF16, tag="bw", name="bw")
        nc.sync.dma_start(bw[:], bigw_d[:])
        bf = pers.tile([128, NF32], F32, tag="bf", name="bf")
        nc.sync.dma_start(bf[:], bigf_d[:])
        zwc = pers.tile([128, B], BF16, tag="zwc", name="zwc")
        for p in range(4):
            nc.sync.dma_start(zwc[:, p * 1024:(p + 1) * 1024],
                              zwc_d[:, p * 1024:(p + 1) * 1024])
        br = pers.tile([128, NR], F32R, tag="br", name="br")
        nc.sync.dma_start(br[:], bigr_d[:])
        # V2 = [cT ; pooled0|pooled1]: loop rhs AND diagonal source
        V2 = pers.tile([128, B], F32R, tag="V2", name="V2")
        nc.sync.dma_start(V2[0:64, :], cT_d[:])

        zwoT = br[:, ZWOF:ZWOF + 512]
        sel2 = bw[0:2, SEL:SEL + 128]
        linb2 = bf[:, LINB:LINB + 1]
        a1b2 = bf[0:64, A1B:A1B + 1]
        a2b2 = bf[0:2, A2B:A2B + 1]

        ztT2 = pers.tile([128, G], BF16, tag="ztT2")
        hT2 = pers.tile([64, G], BF16, tag="hT2")
        eT2 = pers.tile([2, G], BF16, tag="eT2")
        d2 = pers.tile([2, G], BF16, tag="d2")
        bT2 = pers.tile([2, G], BF16, tag="bT2")
        ztwT2 = pers.tile([128, G], F32, tag="ztwT2")
        pooled2 = pers.tile([128, G], F32R, tag="pooled2")
        Sacc = pers.tile([2, 4], F32, tag="Sacc")
        Tacc = pers.tile([128, 4], F32, tag="Tacc")
        T2 = pers.tile([128, 1], F32, tag="T2")
        S2 = pers.tile([2, 1], F32, tag="S2")
        biasS = pers.tile([128, 8], F32, tag="biasS")
        U0 = pers.tile([128, OWN], F32R, tag="U0")
        U1 = pers.tile([128, OWN], F32R, tag="U1")
        UOwnS = pers.tile([128, OWN], F32, tag="UOwnS")
        vout = pers.tile([128, 12], F32, tag="vout")

        actwarm = pers.tile([2, 1], BF16, tag="actwarm")

        with tc.tile_pool(name="psA", bufs=6, space="PSUM") as psA, \
             tc.tile_pool(name="psB", bufs=1, space="PSUM") as psB:
            psU = psA
            # trigger the one-time activation table load before tanh needs it
            nc.scalar.activation(actwarm[:], bw[0:2, 0:1], AF.Tanh)

            # ---- zt -> h -> s -> e chain, issued breadth-first so each
            # engine's in-order queue never blocks the next chunk ----
            SL = [slice(ch * 512, (ch + 1) * 512) for ch in range(4)]
            pzs = []
            for ch in range(4):
                pz = psA.tile([128, 512], F32, tag="ps")
                nc.tensor.matmul(pz[0:64, :], bw[:, LW0:LW0 + 64],
                                 zwc[:, ch * 1024:ch * 1024 + 512],
                                 start=True, stop=True)
                nc.tensor.matmul(pz[64:128, :], bw[:, LW1:LW1 + 64],
                                 zwc[:, ch * 1024 + 512:(ch + 1) * 1024],
                                 start=True, stop=True)
                pzs.append(pz)
            for ch in range(4):
                # relu(x + bias) on DVE (GPSIMD can't read PSUM)
                nc.vector.tensor_scalar(ztT2[:, SL[ch]], pzs[ch][:], linb2, 0.0,
                                        op0=ALU.add, op1=ALU.max)
            phs = []
            for ch in range(4):
                ph = psA.tile([128, 512], F32, tag="ps")
                nc.tensor.matmul(ph[0:64, :], bw[:, A1W:A1W + 64],
                                 ztT2[:, SL[ch]], start=True, stop=True)
                phs.append(ph)
            pss = []
            for ch in range(4):
                ps_ = psA.tile([128, 512], F32, tag="ps")
                pss.append(ps_)
            for ch in range(4):
                nc.scalar.activation(hT2[:, SL[ch]], phs[ch][0:64, :], AF.Tanh,
                                     bias=a1b2)
                nc.tensor.matmul(pss[ch][0:2, :], bw[0:64, A2W:A2W + 2],
                                 hT2[:, SL[ch]], start=True, stop=True)
                nc.scalar.activation(eT2[:, SL[ch]], pss[ch][0:2, :], AF.Exp,
                                     bias=a2b2, accum_out=Sacc[:, ch:ch + 1])

            # ---- e-broadcast, ztw on DVE, T-partials on Act (accum) ----
            pebs = []
            for ch in range(4):
                peb = psA.tile([128, 512], F32, tag="ps")
                nc.tensor.matmul(peb[:], sel2, eT2[:, SL[ch]],
                                 start=True, stop=True)
                pebs.append(peb)
            for ch in range(4):
                nc.vector.tensor_tensor(ztwT2[:, SL[ch]], ztT2[:, SL[ch]],
                                        pebs[ch][:], op=ALU.mult)
                tjunk = scr.tile([128, 512], BF16, tag="tj")
                nc.scalar.activation(tjunk[:], ztwT2[:, SL[ch]], AF.Copy,
                                     accum_out=Tacc[:, ch:ch + 1])

            # ---- beta = 1/(e - S) on DVE ----
            nc.vector.reduce_sum(S2[:], Sacc[:], axis=mybir.AxisListType.X)
            nc.vector.tensor_scalar(d2[:], eT2[:], S2[:], None,
                                    op0=ALU.subtract)
            for ch in range(4):
                with nc.allow_low_precision(reason="beta in bf16 for PE bcast"):
                    nc.vector.reciprocal(bT2[:, SL[ch]], d2[:, SL[ch]])
            nc.vector.reduce_sum(T2[:], Tacc[:], axis=mybir.AxisListType.X)

            # ---- U tiles + delta biases (PE slack while beta computes) ----
            upus = []
            for uoff in (CU0, CU1):
                pu = psA.tile([128, 512], F32, tag="ps")
                nc.tensor.matmul(pu[:], br[:, uoff:uoff + 128],
                                 zwoT, start=True, stop=True)
                upus.append(pu)
            puo = psA.tile([128, 512], F32, tag="ps")
            nc.tensor.matmul(puo[:], br[:, CUWO:CUWO + 128],
                             zwoT, start=True, stop=True)
            nc.scalar.copy(UOwnS[:], puo[:])
            pbias = psB.tile([128, 8], F32, tag="pb")
            for ic in range(4):
                nc.tensor.matmul(pbias[:, ic * 2:ic * 2 + 2],
                                 bw[:, ZWO + ic * 128:ZWO + (ic + 1) * 128],
                                 bw[:, BSTK:BSTK + 2], start=True, stop=True)
            nc.vector.tensor_scalar(biasS[:], pbias[:], -SHIFT, None,
                                    op0=ALU.add)

            # ---- pooled = (ztw - T) * beta_bcast ----
            pbbs = []
            for ch in range(4):
                pbb = psA.tile([128, 512], F32, tag="ps")
                nc.tensor.matmul(pbb[:], sel2, bT2[:, SL[ch]],
                                 start=True, stop=True)
                pbbs.append(pbb)
            for ch in range(4):
                nc.vector.scalar_tensor_tensor(
                    out=pooled2[:, SL[ch]], in0=ztwT2[:, SL[ch]], scalar=T2[:],
                    in1=pbbs[ch][:], op0=ALU.subtract, op1=ALU.mult)
                # V2 assembly piece-DMAs: group1 (loop runs it first), group0
                nc.sync.dma_start(V2[64:128, G + ch * 512:G + (ch + 1) * 512],
                                  pooled2[64:128, SL[ch]])
                nc.sync.dma_start(V2[64:128, SL[ch]], pooled2[0:64, SL[ch]])
            # paced junk matmuls: keep the PE out of its low p-state between
            # the prep matmuls and the main loop
            for ch in range(4):
                pw = psA.tile([128, 512], F32, tag="ps")
                nc.tensor.matmul(pw[:], br[:, CU0:CU0 + 128],
                                 pooled2[:, SL[ch]], start=True, stop=True)

            # U copies last on DVE: needed only once the loop starts
            nc.vector.tensor_copy(U1[:], upus[1][:])
            nc.vector.tensor_copy(U0[:], upus[0][:])

        # ---- main loop: 8 chunks of [128 own rows, 2048 group cols] ----
        with tc.tile_pool(name="pbig", bufs=2, space="PSUM") as pbig:
            for g in (1, 0):
                Ug = U0 if g == 0 else U1
                for ic in range(4):
                    usl = slice(ic * 128, (ic + 1) * 128)
                    pm = pbig.tile([128, G], F32, tag="pb")
                    for q in range(4):
                        qs = slice(q * 512, (q + 1) * 512)
                        nc.tensor.matmul(pm[:, qs], Ug[:, usl],
                                         V2[:, g * G + q * 512:
                                            g * G + (q + 1) * 512],
                                         start=True, stop=True)
                    es = scr.tile([128, G], BF16, tag="es")
                    cc = g * 4 + ic
                    nc.scalar.activation(es[:], pm[:], AF.Exp,
                                         bias=biasS[:, 2 * ic + g:
                                                    2 * ic + g + 1],
                                         accum_out=vout[:, cc:cc + 1])

        # ---- diagonal: diag[i] = UOwn[:,i]·V2[:,own(i)] + delta ----
        with tc.tile_pool(name="ptail", bufs=1, space="PSUM") as pt:
            if static_diag:
                csl = slice(0, OWN)
            else:
                pid = nc.vector.partition_id()
                csl = bass.ts(pid, OWN)
            prod = pers.tile([128, OWN], BF16, tag="prod")
            nc.vector.tensor_tensor(prod[0:64, :], UOwnS[0:64, :],
                                    V2[0:64, csl].bitcast(F32), op=ALU.mult)
            nc.vector.tensor_tensor(prod[64:128, :], UOwnS[64:128, :],
                                    V2[64:128, csl].bitcast(F32),
                                    op=ALU.mult)
            pdg = pt.tile([128, 4], F32, tag="pt")
            for ic in range(4):
                nc.tensor.matmul(pdg[:, ic:ic + 1],
                                 prod[:, ic * 128:(ic + 1) * 128],
                                 bw[:, BONE:BONE + 1], start=True, stop=False)
                nc.tensor.matmul(pdg[:, ic:ic + 1],
                                 bw[:, ZWO + ic * 128:ZWO + (ic + 1) * 128],
                                 bw[:, BOWN:BOWN + 1], start=False, stop=True)
            nc.vector.tensor_copy(vout[:, 8:12], pdg[:])
            nc.sync.dma_start(vout_d[:], vout[:])

    nc.compile()
    return nc


_built = None


def _get_program():
    global _built
    if _built is None:
        _built = _build_program()
    return _built


def make_in_maps(inputs):
    import ml_dtypes
    BF = ml_dtypes.bfloat16
    f = lambda x: np.asarray(x, dtype=np.float32)

    zw = np.concatenate([f(inputs['zw_0']), f(inputs['zw_1'])], axis=0)
    zwT = np.ascontiguousarray(zw.T)                  # [128, 4096]
    # chunk-major layout: block ch = [group0 cols ch*512.. | group1 cols ...]
    zwc = np.empty_like(zwT)
    for ch in range(4):
        zwc[:, ch * 1024:ch * 1024 + 512] = zwT[:, ch * 512:(ch + 1) * 512]
        zwc[:, ch * 1024 + 512:(ch + 1) * 1024] = \
            zwT[:, G + ch * 512:G + (ch + 1) * 512]
    b0 = f(inputs['Ww0_b']) + f(inputs['Wk_b'])
    b1 = f(inputs['Ww1_b']) + f(inputs['Wk_b'])
    wk = f(inputs['Wk_w'])
    uw0 = np.hstack([wk, f(inputs['Ww0_w'])])          # [Czw ; Azw0]
    uw1 = np.hstack([wk, f(inputs['Ww1_w'])])          # [Czw ; Azw1]

    bigw = np.zeros((128, WN), np.float32)
    bigw[:, LW0:LW0 + 64] = f(inputs['lin0_w']).T
    bigw[:, LW1:LW1 + 64] = f(inputs['lin1_w']).T
    bigw[0:64, A1W:A1W + 32] = f(inputs['a0_1w']).T
    bigw[64:128, A1W + 32:A1W + 64] = f(inputs['a1_1w']).T
    bigw[0:32, A2W:A2W + 1] = f(inputs['a0_2w']).T
    bigw[32:64, A2W + 1:A2W + 2] = f(inputs['a1_2w']).T
    bigw[0, SEL:SEL + 64] = 1.0
    bigw[1, SEL + 64:SEL + 128] = 1.0
    bigw[:, BSTK] = b0
    bigw[:, BSTK + 1] = b1
    bigw[:, BONE] = 1.0

    bigr = np.zeros((128, NR), np.float32)
    bigr[:, CU0:CU0 + 128] = uw0
    bigr[:, CU1:CU1 + 128] = uw1
    bigf = np.zeros((128, NF32), np.float32)
    bigf[:, LINB] = np.concatenate([f(inputs['lin0_b']), f(inputs['lin1_b'])])
    bigf[0:64, A1B] = np.concatenate([f(inputs['a0_1b']), f(inputs['a1_1b'])])
    bigf[0:2, A2B] = np.concatenate([f(inputs['a0_2b']), f(inputs['a1_2b'])])

    cT = np.ascontiguousarray(f(inputs['c']).T)        # [64, 4096]

    in_maps = []
    for cid in range(N_CORES):
        g = cid // 4
        mw = bigw.copy()
        mw[:, ZWO:ZWO + OWN] = zwT[:, cid * OWN:(cid + 1) * OWN]
        mw[:, BOWN] = b0 if g == 0 else b1
        mr = bigr.copy()
        mr[:, ZWOF:ZWOF + OWN] = zwT[:, cid * OWN:(cid + 1) * OWN]
        mr[:, CUWO:CUWO + 128] = uw0 if g == 0 else uw1
        in_maps.append({
            'bigw': np.ascontiguousarray(mw.astype(BF)),
            'zwc': np.ascontiguousarray(zwc.astype(BF)),
            'bigr': np.ascontiguousarray(mr),
            'bigf': bigf,
            'cT': cT,
        })
    return in_maps


def kernel(**inputs):
    nc = _get_program()
    in_maps = make_in_maps(inputs)
    res = run_bass_kernel_spmd(nc, in_maps, list(range(N_CORES)))
    tot = 0.0
    for r in res.results:
        v = np.asarray(r['vout'], dtype=np.float64)
        se = v[:, 0:4] + v[:, 4:8]          # [128, 4]: sum over both groups
        dg = v[:, 8:12]
        tot += np.sum(dg - SHIFT - np.log(se))
    return np.array(-(tot / B), dtype=np.float32)


# revision 28
# speedup vs baseline: 1.2623x; 1.0341x over previous
"""Trainium2 Bass kernel for nn_CPCModel (CPC-style NCE loss).

Strategy (8 NeuronCores, full inputs on every core, no collectives):

The reference's leave-one-out softmax pooling collapses algebraically:
    pooled[i] = (T - e_i * zt_i) / (S - e_i),  e = exp(s), S = sum(e), T = sum(e_j zt_j)
so the [B,B] pooling matrix is never materialized.  The loss needs only
    nce = -mean_i( total[i,i] - logsumexp_j total[i,j] )
with  total[i, j in group g] = Azw_g[i]·pooled_g[j] + Czw[i]·c[j] + delta_g[i]
where Azw_g = zw @ Ww_g, Czw = zw @ Wk_w, delta_g = zw @ (Ww_g_b + Wk_b).

Each core redundantly computes the cheap pooling prep for all 4096 rows
(both groups stacked on the 128 partitions) and its own 512 rows of the
[4096,4096] total matrix + row-wise sum(exp(total - 44)) via 8
[128,2048] exp chunks on the scalar engine with accum_out.  The loop
rhs V2 = [cT ; pooled0|pooled1] is assembled by three DMAs (no compute)
and doubles as the diagonal's source through one partition_id-dynamic
column slice.  Raw per-row exp sums and raw diagonal values ship to the
host, which finishes with log() in float64.

Scheduling: weights arrive in a small first DMA and zw^T in two
chunk-major halves so the zt->h->s chain starts ~4us in; exp-of-s is
chunked so the fused ztw/T tensor_tensor_reduce starts before the last
tanh; U/bias matmuls and paced PE probes keep the tensor engine out of
its low p-state before the main loop.  fp32r matmuls keep K=128
throughout (K=64 fp32r dies at runtime).  One activation-table set
(exp_and_others: relu/tanh/exp/copy) serves the whole kernel; the final
log runs on the host.
"""

import numpy as np

import concourse.bacc as bacc
import concourse.bass as bass
import concourse.mybir as mybir
import concourse.tile as tile
from concourse.bass_utils import run_bass_kernel_spmd

N_CORES = 8
B = 4096
OWN = B // N_CORES            # 512 rows of `total` per core
G = 2048                      # group size
F32 = mybir.dt.float32
F32R = mybir.dt.float32r
BF16 = mybir.dt.bfloat16
AF = mybir.ActivationFunctionType
ALU = mybir.AluOpType
SHIFT = 44.0

# bf16 weights bundle column offsets
ZWO = 0           # zwoTb [128, 512]
LW0 = 512         # lin0_w.T [128, 64]
LW1 = 576         # lin1_w.T [128, 64]
A1W = 640         # blockdiag(a0_1w.T, a1_1w.T) [128, 64]
A2W = 704         # blockdiag(a0_2w.T, a1_2w.T) [64, 2]
SEL = 706         # sel2 [2, 128]
BSTK = 834        # [b0 | b1] [128, 2]
BOWN = 836        # b_{group(core)} [128, 1]
BONE = 837        # ones [128, 1]
WN = 838

# f32r bundle column offsets
ZWOF = 0          # zwoT [128, 512]
CU0 = 512         # hstack(Wk_w, Ww0_w) [128, 128]
CU1 = 640         # hstack(Wk_w, Ww1_w)
CUWO = 768        # per-core hstack(Wk_w, Ww_g) [128, 128] (diag)
NR = 896
# f32 bundle column offsets
LINB = 0          # [lin0_b ; lin1_b] [128, 1]
A1B = 1           # [a0_1b ; a1_1b] [64, 1]
A2B = 2           # [a0_2b ; a1_2b] [2, 1]
NF32 = 3


def _build_program(static_diag=False):
    nc = bacc.Bacc(
        "TRN2",
        target_bir_lowering=False,
        debug=False,
        num_devices=N_CORES,
    )

    bigw_d = nc.dram_tensor("bigw", [128, WN], BF16, kind="ExternalInput").ap()
    zwc_d = nc.dram_tensor("zwc", [128, B], BF16, kind="ExternalInput").ap()
    bigr_d = nc.dram_tensor("bigr", [128, NR], F32R, kind="ExternalInput").ap()
    bigf_d = nc.dram_tensor("bigf", [128, NF32], F32, kind="ExternalInput").ap()
    cT_d = nc.dram_tensor("cT", [64, B], F32R, kind="ExternalInput").ap()
    vout_d = nc.dram_tensor("vout", [128, 12], F32, kind="ExternalOutput").ap()

    from contextlib import ExitStack
    with tile.TileContext(nc) as tc, ExitStack() as ctx:
        pers = ctx.enter_context(tc.tile_pool(name="pers", bufs=1))
        scr = ctx.enter_context(tc.tile_pool(name="scr", bufs=2))

        # DMA order = need order: weights, zw halves, f32r/f32 bundles, cT
        bw = pers.tile([128, WN], BF16, tag="bw", name="bw")
        nc.sync.dma_start(bw[:], bigw_d[:])
        bf = pers.tile([128, NF32], F32, tag="bf", name="bf")
        nc.sync.dma_start(bf[:], bigf_d[:])
        zwc = pers.tile([128, B], BF16, tag="zwc", name="zwc")
        for p in range(4):
            nc.sync.dma_start(zwc[:, p * 1024:(p + 1) * 1024],
                              zwc_d[:, p * 1024:(p + 1) * 1024])
        br = pers.tile([128, NR], F32R, tag="br", name="br")
        nc.sync.dma_start(br[:], bigr_d[:])
        # V2 = [cT ; pooled0|pooled1]: loop rhs AND diagonal source
        V2 = pers.tile([128, B], F32R, tag="V2", name="V2")
        nc.sync.dma_start(V2[0:64, :], cT_d[:])

        zwoT = br[:, ZWOF:ZWOF + 512]
        sel2 = bw[0:2, SEL:SEL + 128]
        linb2 = bf[:, LINB:LINB + 1]
        a1b2 = bf[0:64, A1B:A1B + 1]
        a2b2 = bf[0:2, A2B:A2B + 1]

        ztT2 = pers.tile([128, G], BF16, tag="ztT2")
        hT2 = pers.tile([64, G], BF16, tag="hT2")
        eT2 = pers.tile([2, G], BF16, tag="eT2")
        d2 = pers.tile([2, G], BF16, tag="d2")
        bT2 = pers.tile([2, G], BF16, tag="bT2")
        ztwT2 = pers.tile([128, G], F32, tag="ztwT2")
        pooled2 = pers.tile([128, G], F32R, tag="pooled2")
        Sacc = pers.tile([2, 4], F32, tag="Sacc")
        Tacc = pers.tile([128, 4], F32, tag="Tacc")
        T2 = pers.tile([128, 1], F32, tag="T2")
        S2 = pers.tile([2, 1], F32, tag="S2")
        biasS = pers.tile([128, 8], F32, tag="biasS")
        U0 = pers.tile([128, OWN], F32R, tag="U0")
        U1 = pers.tile([128, OWN], F32R, tag="U1")
        UOwnS = pers.tile([128, OWN], F32, tag="UOwnS")
        vout = pers.tile([128, 12], F32, tag="vout")

        actwarm = pers.tile([2, 1], BF16, tag="actwarm")

        with tc.tile_pool(name="psA", bufs=6, space="PSUM") as psA, \
             tc.tile_pool(name="psB", bufs=1, space="PSUM") as psB:
            psU = psA
            # trigger the one-time activation table load before tanh needs it
            nc.scalar.activation(actwarm[:], bw[0:2, 0:1], AF.Tanh)

            # ---- zt -> h -> s -> e chain, issued breadth-first so each
            # engine's in-order queue never blocks the next chunk ----
            SL = [slice(ch * 512, (ch + 1) * 512) for ch in range(4)]
            pzs = []
            for ch in range(4):
                pz = psA.tile([128, 512], F32, tag="ps")
                nc.tensor.matmul(pz[0:64, :], bw[:, LW0:LW0 + 64],
                                 zwc[:, ch * 1024:ch * 1024 + 512],
                                 start=True, stop=True)
                nc.tensor.matmul(pz[64:128, :], bw[:, LW1:LW1 + 64],
                                 zwc[:, ch * 1024 + 512:(ch + 1) * 1024],
                                 start=True, stop=True)
                pzs.append(pz)
            for ch in range(4):
                # relu(x + bias): split across Act and DVE
                if ch % 2 == 0:
                    nc.scalar.activation(ztT2[:, SL[ch]], pzs[ch][:], AF.Relu,
                                         bias=linb2)
                else:
                    nc.vector.tensor_scalar(ztT2[:, SL[ch]], pzs[ch][:], linb2,
                                            0.0, op0=ALU.add, op1=ALU.max)
            phs = []
            for ch in range(4):
                ph = psA.tile([128, 512], F32, tag="ps")
                nc.tensor.matmul(ph[0:64, :], bw[:, A1W:A1W + 64],
                                 ztT2[:, SL[ch]], start=True, stop=True)
                phs.append(ph)
            pss = []
            for ch in range(4):
                ps_ = psA.tile([128, 512], F32, tag="ps")
                pss.append(ps_)
            for ch in range(4):
                nc.scalar.activation(hT2[:, SL[ch]], phs[ch][0:64, :], AF.Tanh,
                                     bias=a1b2)
                nc.tensor.matmul(pss[ch][0:2, :], bw[0:64, A2W:A2W + 2],
                                 hT2[:, SL[ch]], start=True, stop=True)
                nc.scalar.activation(eT2[:, SL[ch]], pss[ch][0:2, :], AF.Exp,
                                     bias=a2b2, accum_out=Sacc[:, ch:ch + 1])

            # ---- e-broadcast, ztw on DVE, T-partials on Act (accum) ----
            pebs = []
            for ch in range(4):
                peb = psA.tile([128, 512], F32, tag="ps")
                nc.tensor.matmul(peb[:], sel2, eT2[:, SL[ch]],
                                 start=True, stop=True)
                pebs.append(peb)
            for ch in range(4):
                nc.vector.tensor_tensor(ztwT2[:, SL[ch]], ztT2[:, SL[ch]],
                                        pebs[ch][:], op=ALU.mult)
                tjunk = scr.tile([128, 512], BF16, tag="tj")
                nc.scalar.activation(tjunk[:], ztwT2[:, SL[ch]], AF.Copy,
                                     accum_out=Tacc[:, ch:ch + 1])

            # ---- beta = 1/(e - S) on DVE ----
            nc.vector.reduce_sum(S2[:], Sacc[:], axis=mybir.AxisListType.X)
            nc.vector.tensor_scalar(d2[:], eT2[:], S2[:], None,
                                    op0=ALU.subtract)

            # ---- U tiles + delta biases (PE slack while beta computes) ----
            upus = []
            for uoff in (CU0, CU1):
                pu = psA.tile([128, 512], F32, tag="ps")
                nc.tensor.matmul(pu[:], br[:, uoff:uoff + 128],
                                 zwoT, start=True, stop=True)
                upus.append(pu)
            puo = psA.tile([128, 512], F32, tag="ps")
            nc.tensor.matmul(puo[:], br[:, CUWO:CUWO + 128],
                             zwoT, start=True, stop=True)
            nc.scalar.copy(UOwnS[:], puo[:])
            pbias = psB.tile([128, 8], F32, tag="pb")
            for ic in range(4):
                nc.tensor.matmul(pbias[:, ic * 2:ic * 2 + 2],
                                 bw[:, ZWO + ic * 128:ZWO + (ic + 1) * 128],
                                 bw[:, BSTK:BSTK + 2], start=True, stop=True)
            nc.vector.tensor_scalar(biasS[:], pbias[:], -SHIFT, None,
                                    op0=ALU.add)

            # ---- beta chunks, T2, pooled combines: interleaved on DVE so
            # the pooled chain starts as soon as T2 and the first beta land
            def recip(ch):
                with nc.allow_low_precision(reason="beta in bf16 for PE bcast"):
                    nc.vector.reciprocal(bT2[:, SL[ch]], d2[:, SL[ch]])

            def bcast(ch):
                pbb = psA.tile([128, 512], F32, tag="ps", name=f"pbb{ch}")
                nc.tensor.matmul(pbb[:], sel2, bT2[:, SL[ch]],
                                 start=True, stop=True)
                return pbb

            def combine(ch, pbb):
                nc.vector.scalar_tensor_tensor(
                    out=pooled2[:, SL[ch]], in0=ztwT2[:, SL[ch]], scalar=T2[:],
                    in1=pbb[:], op0=ALU.subtract, op1=ALU.mult)

            recip(0)
            pbb0 = bcast(0)
            recip(1)
            pbb1 = bcast(1)
            nc.vector.reduce_sum(T2[:], Tacc[:], axis=mybir.AxisListType.X)
            combine(0, pbb0)
            recip(2)
            pbb2 = bcast(2)
            combine(1, pbb1)
            # group-1 half of V2 first: the loop runs group 1 first
            nc.sync.dma_start(V2[64:128, G:G + 1024], pooled2[64:128, 0:1024])
            nc.sync.dma_start(V2[64:128, 0:1024], pooled2[0:64, 0:1024])
            recip(3)
            pbb3 = bcast(3)
            combine(2, pbb2)
            combine(3, pbb3)
            nc.sync.dma_start(V2[64:128, G + 1024:B], pooled2[64:128, 1024:G])
            nc.sync.dma_start(V2[64:128, 1024:G], pooled2[0:64, 1024:G])
            # paced junk matmuls: keep the PE out of its low p-state between
            # the prep matmuls and the main loop
            for ch in range(4):
                pw = psA.tile([128, 512], F32, tag="ps")
                nc.tensor.matmul(pw[:], br[:, CU0:CU0 + 128],
                                 pooled2[:, SL[ch]], start=True, stop=True)

            # U copies last on DVE: needed only once the loop starts
            nc.vector.tensor_copy(U1[:], upus[1][:])
            nc.vector.tensor_copy(U0[:], upus[0][:])

        # ---- main loop: 8 chunks of [128 own rows, 2048 group cols] ----
        with tc.tile_pool(name="pbig", bufs=2, space="PSUM") as pbig:
            for g in (1, 0):
                Ug = U0 if g == 0 else U1
                for ic in range(4):
                    usl = slice(ic * 128, (ic + 1) * 128)
                    pm = pbig.tile([128, G], F32, tag="pb")
                    for q in range(4):
                        qs = slice(q * 512, (q + 1) * 512)
                        nc.tensor.matmul(pm[:, qs], Ug[:, usl],
                                         V2[:, g * G + q * 512:
                                            g * G + (q + 1) * 512],
                                         start=True, stop=True)
                    es = scr.tile([128, G], BF16, tag="es")
                    cc = g * 4 + ic
                    nc.scalar.activation(es[:], pm[:], AF.Exp,
                                         bias=biasS[:, 2 * ic + g:
                                                    2 * ic + g + 1],
                                         accum_out=vout[:, cc:cc + 1])

        # ---- diagonal: diag[i] = UOwn[:,i]·V2[:,own(i)] + delta ----
        with tc.tile_pool(name="ptail", bufs=1, space="PSUM") as pt:
            if static_diag:
                csl = slice(0, OWN)
            else:
                pid = nc.vector.partition_id()
                csl = bass.ts(pid, OWN)
            prod = pers.tile([128, OWN], BF16, tag="prod")
            nc.vector.tensor_tensor(prod[0:64, :], UOwnS[0:64, :],
                                    V2[0:64, csl].bitcast(F32), op=ALU.mult)
            nc.vector.tensor_tensor(prod[64:128, :], UOwnS[64:128, :],
                                    V2[64:128, csl].bitcast(F32),
                                    op=ALU.mult)
            pdg = pt.tile([128, 4], F32, tag="pt")
            for ic in range(4):
                nc.tensor.matmul(pdg[:, ic:ic + 1],
                                 prod[:, ic * 128:(ic + 1) * 128],
                                 bw[:, BONE:BONE + 1], start=True, stop=False)
                nc.tensor.matmul(pdg[:, ic:ic + 1],
                                 bw[:, ZWO + ic * 128:ZWO + (ic + 1) * 128],
                                 bw[:, BOWN:BOWN + 1], start=False, stop=True)
            nc.vector.tensor_copy(vout[:, 8:12], pdg[:])
            nc.sync.dma_start(vout_d[:], vout[:])

    nc.compile()
    return nc


_built = None


def _get_program():
    global _built
    if _built is None:
        _built = _build_program()
    return _built


def make_in_maps(inputs):
    import ml_dtypes
    BF = ml_dtypes.bfloat16
    f = lambda x: np.asarray(x, dtype=np.float32)

    zw = np.concatenate([f(inputs['zw_0']), f(inputs['zw_1'])], axis=0)
    zwT = np.ascontiguousarray(zw.T)                  # [128, 4096]
    # chunk-major layout: block ch = [group0 cols ch*512.. | group1 cols ...]
    zwc = np.empty_like(zwT)
    for ch in range(4):
        zwc[:, ch * 1024:ch * 1024 + 512] = zwT[:, ch * 512:(ch + 1) * 512]
        zwc[:, ch * 1024 + 512:(ch + 1) * 1024] = \
            zwT[:, G + ch * 512:G + (ch + 1) * 512]
    b0 = f(inputs['Ww0_b']) + f(inputs['Wk_b'])
    b1 = f(inputs['Ww1_b']) + f(inputs['Wk_b'])
    wk = f(inputs['Wk_w'])
    uw0 = np.hstack([wk, f(inputs['Ww0_w'])])          # [Czw ; Azw0]
    uw1 = np.hstack([wk, f(inputs['Ww1_w'])])          # [Czw ; Azw1]

    bigw = np.zeros((128, WN), np.float32)
    bigw[:, LW0:LW0 + 64] = f(inputs['lin0_w']).T
    bigw[:, LW1:LW1 + 64] = f(inputs['lin1_w']).T
    bigw[0:64, A1W:A1W + 32] = f(inputs['a0_1w']).T
    bigw[64:128, A1W + 32:A1W + 64] = f(inputs['a1_1w']).T
    bigw[0:32, A2W:A2W + 1] = f(inputs['a0_2w']).T
    bigw[32:64, A2W + 1:A2W + 2] = f(inputs['a1_2w']).T
    bigw[0, SEL:SEL + 64] = 1.0
    bigw[1, SEL + 64:SEL + 128] = 1.0
    bigw[:, BSTK] = b0
    bigw[:, BSTK + 1] = b1
    bigw[:, BONE] = 1.0

    bigr = np.zeros((128, NR), np.float32)
    bigr[:, CU0:CU0 + 128] = uw0
    bigr[:, CU1:CU1 + 128] = uw1
    bigf = np.zeros((128, NF32), np.float32)
    bigf[:, LINB] = np.concatenate([f(inputs['lin0_b']), f(inputs['lin1_b'])])
    bigf[0:64, A1B] = np.concatenate([f(inputs['a0_1b']), f(inputs['a1_1b'])])
    bigf[0:2, A2B] = np.concatenate([f(inputs['a0_2b']), f(inputs['a1_2b'])])

    cT = np.ascontiguousarray(f(inputs['c']).T)        # [64, 4096]

    in_maps = []
    for cid in range(N_CORES):
        g = cid // 4
        mw = bigw.copy()
        mw[:, ZWO:ZWO + OWN] = zwT[:, cid * OWN:(cid + 1) * OWN]
        mw[:, BOWN] = b0 if g == 0 else b1
        mr = bigr.copy()
        mr[:, ZWOF:ZWOF + OWN] = zwT[:, cid * OWN:(cid + 1) * OWN]
        mr[:, CUWO:CUWO + 128] = uw0 if g == 0 else uw1
        in_maps.append({
            'bigw': np.ascontiguousarray(mw.astype(BF)),
            'zwc': np.ascontiguousarray(zwc.astype(BF)),
            'bigr': np.ascontiguousarray(mr),
            'bigf': bigf,
            'cT': cT,
        })
    return in_maps


def kernel(**inputs):
    nc = _get_program()
    in_maps = make_in_maps(inputs)
    res = run_bass_kernel_spmd(nc, in_maps, list(range(N_CORES)))
    tot = 0.0
    for r in res.results:
        v = np.asarray(r['vout'], dtype=np.float64)
        se = v[:, 0:4] + v[:, 4:8]          # [128, 4]: sum over both groups
        dg = v[:, 8:12]
        tot += np.sum(dg - SHIFT - np.log(se))
    return np.array(-(tot / B), dtype=np.float32)


# revision 29
# speedup vs baseline: 1.2770x; 1.0117x over previous
"""Trainium2 Bass kernel for nn_CPCModel (CPC-style NCE loss).

Strategy (8 NeuronCores, full inputs on every core, no collectives):

The reference's leave-one-out softmax pooling collapses algebraically:
    pooled[i] = (T - e_i * zt_i) / (S - e_i),  e = exp(s), S = sum(e), T = sum(e_j zt_j)
so the [B,B] pooling matrix is never materialized.  The loss needs only
    nce = -mean_i( total[i,i] - logsumexp_j total[i,j] )
with  total[i, j in group g] = Azw_g[i]·pooled_g[j] + Czw[i]·c[j] + delta_g[i]
where Azw_g = zw @ Ww_g, Czw = zw @ Wk_w, delta_g = zw @ (Ww_g_b + Wk_b).

Each core redundantly computes the cheap pooling prep for all 4096 rows
(both groups stacked on the 128 partitions) and its own 512 rows of the
[4096,4096] total matrix + row-wise sum(exp(total - 44)) via 8
[128,2048] exp chunks on the scalar engine with accum_out.  The loop
rhs V2 = [cT ; pooled0|pooled1] is assembled by three DMAs (no compute)
and doubles as the diagonal's source through one partition_id-dynamic
column slice.  Raw per-row exp sums and raw diagonal values ship to the
host, which finishes with log() in float64.

Scheduling: weights arrive in a small first DMA and zw^T in two
chunk-major halves so the zt->h->s chain starts ~4us in; exp-of-s is
chunked so the fused ztw/T tensor_tensor_reduce starts before the last
tanh; U/bias matmuls and paced PE probes keep the tensor engine out of
its low p-state before the main loop.  fp32r matmuls keep K=128
throughout (K=64 fp32r dies at runtime).  One activation-table set
(exp_and_others: relu/tanh/exp/copy) serves the whole kernel; the final
log runs on the host.
"""

import numpy as np

import concourse.bacc as bacc
import concourse.bass as bass
import concourse.mybir as mybir
import concourse.tile as tile
from concourse.bass_utils import run_bass_kernel_spmd

N_CORES = 8
B = 4096
OWN = B // N_CORES            # 512 rows of `total` per core
G = 2048                      # group size
F32 = mybir.dt.float32
F32R = mybir.dt.float32r
BF16 = mybir.dt.bfloat16
AF = mybir.ActivationFunctionType
ALU = mybir.AluOpType
SHIFT = 44.0

# bf16 weights bundle column offsets
ZWO = 0           # zwoTb [128, 512]
LW0 = 512         # lin0_w.T [128, 64]
LW1 = 576         # lin1_w.T [128, 64]
A1W = 640         # blockdiag(a0_1w.T, a1_1w.T) [128, 64]
A2W = 704         # blockdiag(a0_2w.T, a1_2w.T) [64, 2]
SEL = 706         # sel2 [2, 128]
BSTK = 834        # [b0 | b1] [128, 2]
BOWN = 836        # b_{group(core)} [128, 1]
BONE = 837        # ones [128, 1]
WN = 838

# f32r bundle column offsets
ZWOF = 0          # zwoT [128, 512]
CU0 = 512         # hstack(Wk_w, Ww0_w) [128, 128]
CU1 = 640         # hstack(Wk_w, Ww1_w)
CUWO = 768        # per-core hstack(Wk_w, Ww_g) [128, 128] (diag)
NR = 896
# f32 bundle column offsets
LINB = 0          # [lin0_b ; lin1_b] [128, 1]
A1B = 1           # [a0_1b ; a1_1b] [64, 1]
A2B = 2           # [a0_2b ; a1_2b] [2, 1]
NF32 = 3


def _build_program(static_diag=False):
    nc = bacc.Bacc(
        "TRN2",
        target_bir_lowering=False,
        debug=False,
        num_devices=N_CORES,
    )

    bigw_d = nc.dram_tensor("bigw", [128, WN], BF16, kind="ExternalInput").ap()
    zwc_d = nc.dram_tensor("zwc", [128, B], BF16, kind="ExternalInput").ap()
    bigr_d = nc.dram_tensor("bigr", [128, NR], F32R, kind="ExternalInput").ap()
    bigf_d = nc.dram_tensor("bigf", [128, NF32], F32, kind="ExternalInput").ap()
    cT_d = nc.dram_tensor("cT", [64, B], F32R, kind="ExternalInput").ap()
    vout_d = nc.dram_tensor("vout", [128, 20], F32, kind="ExternalOutput").ap()

    from contextlib import ExitStack
    with tile.TileContext(nc) as tc, ExitStack() as ctx:
        pers = ctx.enter_context(tc.tile_pool(name="pers", bufs=1))
        scr = ctx.enter_context(tc.tile_pool(name="scr", bufs=2))

        # DMA order = need order: weights, zw halves, f32r/f32 bundles, cT
        bw = pers.tile([128, WN], BF16, tag="bw", name="bw")
        nc.sync.dma_start(bw[:], bigw_d[:])
        bf = pers.tile([128, NF32], F32, tag="bf", name="bf")
        nc.sync.dma_start(bf[:], bigf_d[:])
        zwc = pers.tile([128, B], BF16, tag="zwc", name="zwc")
        for p in range(4):
            nc.sync.dma_start(zwc[:, p * 1024:(p + 1) * 1024],
                              zwc_d[:, p * 1024:(p + 1) * 1024])
        br = pers.tile([128, NR], F32R, tag="br", name="br")
        nc.sync.dma_start(br[:], bigr_d[:])
        # V2 = [cT ; pooled0|pooled1]: loop rhs AND diagonal source
        V2 = pers.tile([128, B], F32R, tag="V2", name="V2")
        nc.sync.dma_start(V2[0:64, :], cT_d[:])

        zwoT = br[:, ZWOF:ZWOF + 512]
        sel2 = bw[0:2, SEL:SEL + 128]
        linb2 = bf[:, LINB:LINB + 1]
        a1b2 = bf[0:64, A1B:A1B + 1]
        a2b2 = bf[0:2, A2B:A2B + 1]

        ztT2 = pers.tile([128, G], BF16, tag="ztT2")
        hT2 = pers.tile([64, G], BF16, tag="hT2")
        eT2 = pers.tile([2, G], BF16, tag="eT2")
        d2 = pers.tile([2, G], BF16, tag="d2")
        bT2 = pers.tile([2, G], BF16, tag="bT2")
        ztwT2 = pers.tile([128, G], F32, tag="ztwT2")
        pooled2 = pers.tile([128, G], F32R, tag="pooled2")
        Sacc = pers.tile([2, 4], F32, tag="Sacc")
        Tacc = pers.tile([128, 4], F32, tag="Tacc")
        T2 = pers.tile([128, 1], F32, tag="T2")
        S2 = pers.tile([2, 1], F32, tag="S2")
        biasS = pers.tile([128, 8], F32, tag="biasS")
        U0 = pers.tile([128, OWN], F32R, tag="U0")
        U1 = pers.tile([128, OWN], F32R, tag="U1")
        UOwnS = pers.tile([128, OWN], F32, tag="UOwnS")
        vout = pers.tile([128, 20], F32, tag="vout")

        actwarm = pers.tile([2, 1], BF16, tag="actwarm")

        with tc.tile_pool(name="psA", bufs=6, space="PSUM") as psA, \
             tc.tile_pool(name="psB", bufs=1, space="PSUM") as psB:
            psU = psA
            # trigger the one-time activation table load before tanh needs it
            nc.scalar.activation(actwarm[:], bw[0:2, 0:1], AF.Tanh)

            # ---- zt -> h -> s -> e chain, issued breadth-first so each
            # engine's in-order queue never blocks the next chunk ----
            SL = [slice(ch * 512, (ch + 1) * 512) for ch in range(4)]
            pzs = []
            for ch in range(4):
                pz = psA.tile([128, 512], F32, tag="ps")
                nc.tensor.matmul(pz[0:64, :], bw[:, LW0:LW0 + 64],
                                 zwc[:, ch * 1024:ch * 1024 + 512],
                                 start=True, stop=True)
                nc.tensor.matmul(pz[64:128, :], bw[:, LW1:LW1 + 64],
                                 zwc[:, ch * 1024 + 512:(ch + 1) * 1024],
                                 start=True, stop=True)
                pzs.append(pz)
            for ch in range(4):
                # relu(x + bias): split across Act and DVE
                if ch % 2 == 0:
                    nc.scalar.activation(ztT2[:, SL[ch]], pzs[ch][:], AF.Relu,
                                         bias=linb2)
                else:
                    nc.vector.tensor_scalar(ztT2[:, SL[ch]], pzs[ch][:], linb2,
                                            0.0, op0=ALU.add, op1=ALU.max)
            phs = []
            for ch in range(4):
                ph = psA.tile([128, 512], F32, tag="ps")
                nc.tensor.matmul(ph[0:64, :], bw[:, A1W:A1W + 64],
                                 ztT2[:, SL[ch]], start=True, stop=True)
                phs.append(ph)
            pss = []
            for ch in range(4):
                ps_ = psA.tile([128, 512], F32, tag="ps")
                pss.append(ps_)
            for ch in range(4):
                nc.scalar.activation(hT2[:, SL[ch]], phs[ch][0:64, :], AF.Tanh,
                                     bias=a1b2)
                nc.tensor.matmul(pss[ch][0:2, :], bw[0:64, A2W:A2W + 2],
                                 hT2[:, SL[ch]], start=True, stop=True)
                nc.scalar.activation(eT2[:, SL[ch]], pss[ch][0:2, :], AF.Exp,
                                     bias=a2b2, accum_out=Sacc[:, ch:ch + 1])

            # ---- e-broadcast, ztw on DVE, T-partials on Act (accum) ----
            pebs = []
            for ch in range(4):
                peb = psA.tile([128, 512], F32, tag="ps")
                nc.tensor.matmul(peb[:], sel2, eT2[:, SL[ch]],
                                 start=True, stop=True)
                pebs.append(peb)
            for ch in range(4):
                nc.vector.tensor_tensor(ztwT2[:, SL[ch]], ztT2[:, SL[ch]],
                                        pebs[ch][:], op=ALU.mult)
                tjunk = scr.tile([128, 512], BF16, tag="tj")
                nc.scalar.activation(tjunk[:], ztwT2[:, SL[ch]], AF.Copy,
                                     accum_out=Tacc[:, ch:ch + 1])

            # ---- beta = 1/(e - S) on DVE ----
            nc.vector.reduce_sum(S2[:], Sacc[:], axis=mybir.AxisListType.X)
            nc.vector.tensor_scalar(d2[:], eT2[:], S2[:], None,
                                    op0=ALU.subtract)

            # ---- U tiles + delta biases (PE slack while beta computes) ----
            upus = []
            for uoff in (CU0, CU1):
                pu = psA.tile([128, 512], F32, tag="ps")
                nc.tensor.matmul(pu[:], br[:, uoff:uoff + 128],
                                 zwoT, start=True, stop=True)
                upus.append(pu)
            puo = psA.tile([128, 512], F32, tag="ps")
            nc.tensor.matmul(puo[:], br[:, CUWO:CUWO + 128],
                             zwoT, start=True, stop=True)
            nc.scalar.copy(UOwnS[:], puo[:])
            pbias = psB.tile([128, 8], F32, tag="pb")
            for ic in range(4):
                nc.tensor.matmul(pbias[:, ic * 2:ic * 2 + 2],
                                 bw[:, ZWO + ic * 128:ZWO + (ic + 1) * 128],
                                 bw[:, BSTK:BSTK + 2], start=True, stop=True)
            nc.vector.tensor_scalar(biasS[:], pbias[:], -SHIFT, None,
                                    op0=ALU.add)

            # ---- beta chunks, T2, pooled combines: interleaved on DVE so
            # the pooled chain starts as soon as T2 and the first beta land
            def recip(ch):
                with nc.allow_low_precision(reason="beta in bf16 for PE bcast"):
                    nc.vector.reciprocal(bT2[:, SL[ch]], d2[:, SL[ch]])

            def bcast(ch):
                pbb = psA.tile([128, 512], F32, tag="ps", name=f"pbb{ch}")
                nc.tensor.matmul(pbb[:], sel2, bT2[:, SL[ch]],
                                 start=True, stop=True)
                return pbb

            def combine(ch, pbb):
                nc.vector.scalar_tensor_tensor(
                    out=pooled2[:, SL[ch]], in0=ztwT2[:, SL[ch]], scalar=T2[:],
                    in1=pbb[:], op0=ALU.subtract, op1=ALU.mult)

            recip(0)
            pbb0 = bcast(0)
            recip(1)
            pbb1 = bcast(1)
            nc.vector.reduce_sum(T2[:], Tacc[:], axis=mybir.AxisListType.X)
            combine(0, pbb0)
            recip(2)
            pbb2 = bcast(2)
            combine(1, pbb1)
            # group-1 half of V2 first: the loop runs group 1 first
            nc.sync.dma_start(V2[64:128, G:G + 1024], pooled2[64:128, 0:1024])
            nc.sync.dma_start(V2[64:128, 0:1024], pooled2[0:64, 0:1024])
            recip(3)
            pbb3 = bcast(3)
            combine(2, pbb2)
            combine(3, pbb3)
            nc.sync.dma_start(V2[64:128, G + 1024:B], pooled2[64:128, 1024:G])
            nc.sync.dma_start(V2[64:128, 1024:G], pooled2[0:64, 1024:G])
            # paced junk matmuls: keep the PE out of its low p-state between
            # the prep matmuls and the main loop
            for ch in range(4):
                pw = psA.tile([128, 512], F32, tag="ps")
                nc.tensor.matmul(pw[:], br[:, CU0:CU0 + 128],
                                 pooled2[:, SL[ch]], start=True, stop=True)

            # U copies last on DVE: needed only once the loop starts
            nc.vector.tensor_copy(U1[:], upus[1][:])
            nc.vector.tensor_copy(U0[:], upus[0][:])

        # ---- main loop over the [512 own rows, 4096 cols] of `total`:
        # mostly 2048-col exp chunks; the first two group-1 chunks split in
        # 1024-col halves so exp work starts as soon as the first half of
        # pooled1 lands in V2 (the rest of the tail still streaming) ----
        with tc.tile_pool(name="pbig", bufs=2, space="PSUM") as pbig:
            def loop_chunk(g, ic, h, width, cc):
                usl = slice(ic * 128, (ic + 1) * 128)
                Ug = U0 if g == 0 else U1
                base = g * G + h * 1024
                pm = pbig.tile([128, width], F32, tag="pb",
                               padded_shape=[128, G])
                for q in range(width // 512):
                    qs = slice(q * 512, (q + 1) * 512)
                    nc.tensor.matmul(pm[:, qs], Ug[:, usl],
                                     V2[:, base + q * 512:
                                        base + (q + 1) * 512],
                                     start=True, stop=True)
                es = scr.tile([128, width], BF16, tag="es",
                              padded_shape=[128, G])
                nc.scalar.activation(es[:], pm[:], AF.Exp,
                                     bias=biasS[:, 2 * ic + g:2 * ic + g + 1],
                                     accum_out=vout[:, cc:cc + 1])

            loop_chunk(1, 0, 0, 1024, 12)
            loop_chunk(1, 0, 1, 1024, 13)
            loop_chunk(1, 1, 0, 1024, 14)
            loop_chunk(1, 1, 1, 1024, 15)
            for g, ic in [(1, 2), (1, 3), (0, 0), (0, 1), (0, 2), (0, 3)]:
                loop_chunk(g, ic, 0, G, g * 4 + ic)

        # ---- diagonal: diag[i] = UOwn[:,i]·V2[:,own(i)] + delta ----
        with tc.tile_pool(name="ptail", bufs=1, space="PSUM") as pt:
            if static_diag:
                csl = slice(0, OWN)
            else:
                pid = nc.vector.partition_id()
                csl = bass.ts(pid, OWN)
            prod = pers.tile([128, OWN], BF16, tag="prod")
            nc.vector.tensor_tensor(prod[0:64, :], UOwnS[0:64, :],
                                    V2[0:64, csl].bitcast(F32), op=ALU.mult)
            nc.vector.tensor_tensor(prod[64:128, :], UOwnS[64:128, :],
                                    V2[64:128, csl].bitcast(F32),
                                    op=ALU.mult)
            pdg = pt.tile([128, 4], F32, tag="pt")
            for ic in range(4):
                nc.tensor.matmul(pdg[:, ic:ic + 1],
                                 prod[:, ic * 128:(ic + 1) * 128],
                                 bw[:, BONE:BONE + 1], start=True, stop=False)
                nc.tensor.matmul(pdg[:, ic:ic + 1],
                                 bw[:, ZWO + ic * 128:ZWO + (ic + 1) * 128],
                                 bw[:, BOWN:BOWN + 1], start=False, stop=True)
            nc.vector.tensor_copy(vout[:, 16:20], pdg[:])
            nc.sync.dma_start(vout_d[:], vout[:])

    nc.compile()
    return nc


_built = None


def _get_program():
    global _built
    if _built is None:
        _built = _build_program()
    return _built


def make_in_maps(inputs):
    import ml_dtypes
    BF = ml_dtypes.bfloat16
    f = lambda x: np.asarray(x, dtype=np.float32)

    zw = np.concatenate([f(inputs['zw_0']), f(inputs['zw_1'])], axis=0)
    zwT = np.ascontiguousarray(zw.T)                  # [128, 4096]
    # chunk-major layout: block ch = [group0 cols ch*512.. | group1 cols ...]
    zwc = np.empty_like(zwT)
    for ch in range(4):
        zwc[:, ch * 1024:ch * 1024 + 512] = zwT[:, ch * 512:(ch + 1) * 512]
        zwc[:, ch * 1024 + 512:(ch + 1) * 1024] = \
            zwT[:, G + ch * 512:G + (ch + 1) * 512]
    b0 = f(inputs['Ww0_b']) + f(inputs['Wk_b'])
    b1 = f(inputs['Ww1_b']) + f(inputs['Wk_b'])
    wk = f(inputs['Wk_w'])
    uw0 = np.hstack([wk, f(inputs['Ww0_w'])])          # [Czw ; Azw0]
    uw1 = np.hstack([wk, f(inputs['Ww1_w'])])          # [Czw ; Azw1]

    bigw = np.zeros((128, WN), np.float32)
    bigw[:, LW0:LW0 + 64] = f(inputs['lin0_w']).T
    bigw[:, LW1:LW1 + 64] = f(inputs['lin1_w']).T
    bigw[0:64, A1W:A1W + 32] = f(inputs['a0_1w']).T
    bigw[64:128, A1W + 32:A1W + 64] = f(inputs['a1_1w']).T
    bigw[0:32, A2W:A2W + 1] = f(inputs['a0_2w']).T
    bigw[32:64, A2W + 1:A2W + 2] = f(inputs['a1_2w']).T
    bigw[0, SEL:SEL + 64] = 1.0
    bigw[1, SEL + 64:SEL + 128] = 1.0
    bigw[:, BSTK] = b0
    bigw[:, BSTK + 1] = b1
    bigw[:, BONE] = 1.0

    bigr = np.zeros((128, NR), np.float32)
    bigr[:, CU0:CU0 + 128] = uw0
    bigr[:, CU1:CU1 + 128] = uw1
    bigf = np.zeros((128, NF32), np.float32)
    bigf[:, LINB] = np.concatenate([f(inputs['lin0_b']), f(inputs['lin1_b'])])
    bigf[0:64, A1B] = np.concatenate([f(inputs['a0_1b']), f(inputs['a1_1b'])])
    bigf[0:2, A2B] = np.concatenate([f(inputs['a0_2b']), f(inputs['a1_2b'])])

    cT = np.ascontiguousarray(f(inputs['c']).T)        # [64, 4096]

    in_maps = []
    for cid in range(N_CORES):
        g = cid // 4
        mw = bigw.copy()
        mw[:, ZWO:ZWO + OWN] = zwT[:, cid * OWN:(cid + 1) * OWN]
        mw[:, BOWN] = b0 if g == 0 else b1
        mr = bigr.copy()
        mr[:, ZWOF:ZWOF + OWN] = zwT[:, cid * OWN:(cid + 1) * OWN]
        mr[:, CUWO:CUWO + 128] = uw0 if g == 0 else uw1
        in_maps.append({
            'bigw': np.ascontiguousarray(mw.astype(BF)),
            'zwc': np.ascontiguousarray(zwc.astype(BF)),
            'bigr': np.ascontiguousarray(mr),
            'bigf': bigf,
            'cT': cT,
        })
    return in_maps


def kernel(**inputs):
    nc = _get_program()
    in_maps = make_in_maps(inputs)
    res = run_bass_kernel_spmd(nc, in_maps, list(range(N_CORES)))
    tot = 0.0
    for r in res.results:
        v = np.asarray(r['vout'], dtype=np.float64)
        v[:, 4] = v[:, 12] + v[:, 13]       # split chunks (g1, ic0/ic1)
        v[:, 5] = v[:, 14] + v[:, 15]
        se = v[:, 0:4] + v[:, 4:8]          # [128, 4]: sum over both groups
        dg = v[:, 8:12]
        tot += np.sum(dg - SHIFT - np.log(se))
    return np.array(-(tot / B), dtype=np.float32)
